# revision 27
# baseline (speedup 1.0000x reference)
"""Bass/Trainium2 kernel for nn_Net_19602230739296 (NNConv + GRU message passing GNN).

Algorithm (mathematically equivalent to the reference):
  theta[e] = (edge_attr[e] @ nn_w + nn_b).reshape(H, H) is never materialized.
  msg[e]   = sum_c ea'[e,c] * (out[src_e] @ W_c)   with ea' = [edge_attr, 1],
             W_c = nn_w[c].reshape(H,H) for c<4, W_4 = nn_b.reshape(H,H).
  agg^T    = sum_c W_c^T @ (G^T @ Q_c)  per 128-edge tile, where G = out[src]
             (gathered rows) and Q_c[e, slot] = ea'[e,c] * [dst_e == slot-node]
             is a host-precomputed weighted one-hot "scatter" matrix.

Numerics: every fp32 value on the matmul paths is represented as a bf16
hi/lo pair (hi = bf16(x), lo = bf16(x - hi)).  bf16 matmuls run at 1 PE
cycle/row vs fp32's 4, and the PE multiplies bf16 exactly with fp32
accumulation, so a 3-term product (hi*hi + hi*lo + lo*hi) is accurate to
~2^-18 relative -- far inside the 2e-2 harness gate.  Node features live in
DRAM as [node, 128] rows = (hi 64 | lo 64) bf16, so one 256B-row gather
feeds the edge matmul with both terms and the per-tile matmul computes the
hi- and lo- partial products in one pass (128-partition PSUM output).

Sharding: edges are sorted by destination and packed into tiles of <=128
edges covering <=32 whole destination nodes.  Real tiles are dealt
round-robin across the 8 cores so each core gets an equal share of edges.
Nodes are renumbered to (tile*32 + slot).  A core's edges land only in its
own node range, so no cross-core reduction is needed.  The evolving node
features are replicated via AllGather each iteration (chunked, so the
collective overlaps the tail of the GRU); iteration 0's features are
computed for ALL nodes on every core (lin0 is tiny), which removes one
AllGather entirely.
"""
import os
import sys

import numpy as np


def _ensure_path():
    for p in ("/opt/trn_rl_repo", os.path.expanduser("~/.axon_site/_ro/trn_rl_repo")):
        if os.path.isdir(p) and p not in sys.path:
            sys.path.insert(0, p)
    try:
        import concourse  # noqa: F401
    except ImportError as e:  # pragma: no cover
        raise ImportError(f"concourse (bass) not importable: {e}")


_ensure_path()

N_NODES, N_EDGES, IN_F, H = 10000, 50000, 32, 64
NCORES = 8
SLOTS = 24            # destination-node slots per tile
EPT = 128             # edge slots per tile
NCH = 5               # edge_attr channels (4) + constant channel for nn_b
T = 64                # tiles per core (fixed so the compiled NEFF is shape-stable)
NTILES = NCORES * T   # 448
NC_COLS = T * SLOTS   # padded nodes per core (1792)
NPAD = NCORES * NC_COLS
CHUNK = 384
GATHER_CHUNKS = 16
N_SWDGE_QUEUES = 4
QW = NCH * SLOTS      # Q columns per tile
NSRC0 = 6144          # compact it-0 feature-table rows (unique sources, padded)
FB = 2 * H            # 128 bf16 feature bytes-row: hi|lo


def _chunks():
    out = []
    c0 = 0
    while c0 < NC_COLS:
        w = min(CHUNK, NC_COLS - c0)
        out.append((c0, w))
        c0 += w
    return out


# ----------------------------------------------------------------------------
# device program
# ----------------------------------------------------------------------------
_NC_CACHE = {}


def _get_nc():
    if "nc" in _NC_CACHE:
        return _NC_CACHE["nc"]
    import concourse.bacc as bacc
    import concourse.mybir as mybir
    import concourse.tile as tile

    dt = mybir.dt
    f32, i16, bf16 = dt.float32, dt.int16, dt.bfloat16
    AF = mybir.ActivationFunctionType
    ALU = mybir.AluOpType

    nc = bacc.Bacc(
        "TRN2",
        target_bir_lowering=False,
        debug=False,
        enable_asserts=False,
        num_devices=NCORES,
        num_swdge_queues=N_SWDGE_QUEUES,
    )

    qh_in = nc.dram_tensor("qh_in", [128, T * QW], bf16, kind="ExternalInput").ap()
    ql_in = nc.dram_tensor("ql_in", [128, T * QW], bf16, kind="ExternalInput").ap()
    idx_in = nc.dram_tensor("idx_in", [128, T * 8], i16, kind="ExternalInput").ap()
    xs0_in = nc.dram_tensor("xs0_in", [64, NSRC0], bf16, kind="ExternalInput").ap()
    idx0_in = nc.dram_tensor("idx0_in", [128, T * 8], i16, kind="ExternalInput").ap()
    xso_in = nc.dram_tensor("xso_in", [64, NC_COLS], bf16, kind="ExternalInput").ap()
    l0_in = nc.dram_tensor("l0_in", [64, 128], bf16, kind="ExternalInput").ap()
    ws_in = nc.dram_tensor("ws_in", [128, NCH * 128], bf16, kind="ExternalInput").ap()
    root_in = nc.dram_tensor("root_in", [128, 64], bf16, kind="ExternalInput").ap()
    gru_in = nc.dram_tensor("gru_in", [128, 384], bf16, kind="ExternalInput").ap()
    bias_in = nc.dram_tensor("bias_in", [128, 8], f32, kind="ExternalInput").ap()
    ident_in = nc.dram_tensor("ident_in", [128, 128], bf16, kind="ExternalInput").ap()
    identf_in = nc.dram_tensor("identf_in", [64, 64], f32, kind="ExternalInput").ap()
    out_ext = nc.dram_tensor("out_sl", [NC_COLS, H], f32, kind="ExternalOutput").ap()

    chunks = _chunks()

    with tile.TileContext(nc) as tc:
        with tc.tile_pool(name="const", bufs=1) as const, \
             tc.tile_pool(name="work", bufs=1) as work, \
             tc.tile_pool(name="small", bufs=2) as small, \
             tc.tile_pool(name="ps", bufs=2, space="PSUM") as ps, \
             tc.tile_pool(name="aggp", bufs=2, space="PSUM") as aggp, \
             tc.tile_pool(name="grz", bufs=1, space="PSUM") as grz, \
             tc.tile_pool(name="gn", bufs=2, space="PSUM") as gn, \
             tc.tile_pool(name="tpp", bufs=1, space="PSUM") as tpp, \
             tc.tile_pool(name="dram", bufs=1, space="DRAM") as dram:

            qh_sb = const.tile([128, T * QW], bf16, name="qh_sb")
            ql_sb = const.tile([128, T * QW], bf16, name="ql_sb")
            idx_sb = const.tile([128, T * 8], i16, name="idx_sb")
            xs0_sb = const.tile([64, NSRC0], bf16, name="xs0_sb")
            idx0_sb = const.tile([128, T * 8], i16, name="idx0_sb")
            xso_sb = const.tile([64, NC_COLS], bf16, name="xso_sb")
            l0_sb = const.tile([64, 128], bf16, name="l0_sb")
            ws_sb = const.tile([128, NCH * 128], bf16, name="ws_sb")
            root_sb = const.tile([128, 64], bf16, name="root_sb")
            gru_sb = const.tile([128, 384], bf16, name="gru_sb")
            bias_sb = const.tile([128, 8], f32, name="bias_sb")
            ident_sb = const.tile([128, 128], bf16, name="ident_sb")
            identf_sb = const.tile([64, 64], f32, name="identf_sb")

            for sb_t, in_t in (
                (xso_sb, xso_in), (l0_sb, l0_in), (bias_sb, bias_in),
                (ident_sb, ident_in), (identf_sb, identf_in),
                (xs0_sb, xs0_in), (idx0_sb, idx0_in), (idx_sb, idx_in),
                (gru_sb, gru_in), (root_sb, root_in), (ws_sb, ws_in),
                (qh_sb, qh_in), (ql_sb, ql_in),
            ):
                nc.sync.dma_start(sb_t[:], in_t[:])

            l0b = bias_sb[0:64, 0:1]
            convb = bias_sb[0:64, 1:2]
            brz = bias_sb[:, 2:3]          # [br ; bz] stacked on 128 partitions
            bnih = bias_sb[0:64, 4:5]
            bnhh = bias_sb[0:64, 5:6]

            h32a = work.tile([128, NC_COLS], f32, name="h32a")
            h32b = work.tile([128, NC_COLS], f32, name="h32b")
            # h bf16 split, stacked [hi;lo] and swapped [lo;hi] on 128 parts
            hs1a = work.tile([128, NC_COLS], bf16, name="hs1a")
            hs2a = work.tile([128, NC_COLS], bf16, name="hs2a")
            hs1b = work.tile([128, NC_COLS], bf16, name="hs1b")
            hs2b = work.tile([128, NC_COLS], bf16, name="hs2b")
            g_sb = work.tile([128, T * FB], bf16, name="g_sb")
            sh_sb = work.tile([128, T * QW], bf16, name="sh_sb")
            sl_sb = work.tile([128, T * QW], bf16, name="sl_sb")
            rowb = work.tile([128, NC_COLS], bf16, name="rowb")
            rowf = work.tile([128, (NC_COLS // 128) * H], f32, name="rowf")
            warm_sb = work.tile([128, 64], bf16, name="warm_sb")

            feat0 = dram.tile([NSRC0, FB], bf16, name="feat0")
            agins = [dram.tile([NC_COLS, FB], bf16, name=f"agin{i}") for i in (1, 2)]
            agouts = [
                dram.tile([NPAD, FB], bf16, addr_space="Shared", name=f"agout{i}")
                for i in (1, 2)
            ]

            # ---- iteration 0: out0 = relu(x @ lin0_w + lin0_b) --------------
            # Full-table pass: every core computes out0 for ALL nodes and
            # writes the bf16 hi|lo feature table locally -- no collective.
            for c in range(NSRC0 // CHUNK):
                sl = slice(c * CHUNK, (c + 1) * CHUNK)
                p0 = gn.tile([64, CHUNK], f32, tag="gn", name=f"l0f{c}")
                nc.tensor.matmul(p0[:], l0_sb[:, 0:64], xs0_sb[:, sl],
                                 start=True, stop=False)
                nc.tensor.matmul(p0[:], l0_sb[:, 64:128], xs0_sb[:, sl],
                                 start=False, stop=True)
                o32 = small.tile([128, CHUNK], f32, tag="o32", name=f"o32_{c}")
                nc.scalar.activation(o32[0:64, :], p0[:], AF.Relu, bias=l0b)
                osk = small.tile([128, CHUNK], bf16, tag="osk", name=f"osk_{c}")
                nc.vector.tensor_copy(osk[0:64, :], o32[0:64, :])
                nc.vector.tensor_sub(osk[64:128, :], o32[0:64, :], osk[0:64, :])
                tp = tpp.tile([128, 512], bf16, tag="tp", name=f"t0_{c}")
                for b in range(CHUNK // 128):
                    bs = slice(b * 128, (b + 1) * 128)
                    nc.tensor.transpose(tp[:, b * 128:(b + 1) * 128],
                                        osk[:, bs], ident_sb[:])
                nbc = CHUNK // 128
                rf = small.tile([128, CHUNK], bf16, tag="rf", name=f"rf_{c}")
                if c % 2 == 0:
                    nc.scalar.copy(rf[:], tp[:, :CHUNK])
                else:
                    nc.vector.tensor_copy(rf[:], tp[:, :CHUNK])
                nc.sync.dma_start(
                    feat0[:].rearrange("(t p) f -> p t f", p=128)[
                        :, nbc * c:nbc * c + nbc, :],
                    rf[:].rearrange("p (t f) -> p t f", f=FB),
                )
            # Own-slice pass (fp32 h and its bf16 split for root/GRU inputs).
            for k, (c0, w) in enumerate(chunks):
                sl = slice(c0, c0 + w)
                p1 = gn.tile([64, CHUNK], f32, tag="gn", name=f"l0o{k}")
                nc.tensor.matmul(p1[:, :w], l0_sb[:, 0:64], xso_sb[:, sl],
                                 start=True, stop=False)
                nc.tensor.matmul(p1[:, :w], l0_sb[:, 64:128], xso_sb[:, sl],
                                 start=False, stop=True)
                nc.scalar.activation(h32a[0:64, sl], p1[:, :w], AF.Relu, bias=l0b)
                nc.vector.tensor_copy(hs1a[0:64, sl], h32a[0:64, sl])
                nc.vector.tensor_sub(hs1a[64:128, sl], h32a[0:64, sl],
                                     hs1a[0:64, sl])
                nc.scalar.copy(hs2a[0:64, sl], hs1a[64:128, sl])
                nc.scalar.copy(hs2a[64:128, sl], hs1a[0:64, sl])

            def warm_chain(links, it):
                # Keep the PE's HAM activity monitor from dropping to the
                # cold 1.2GHz clock while the engine waits on the collective:
                # a dependency chain of tiny matmuls paces ~1 PE op per µs.
                for i in range(links):
                    wp = gn.tile([64, CHUNK], f32, tag="gn", name=f"wm{it}_{i}")
                    nc.tensor.matmul(wp[:, 0:64], ident_sb[:, 0:64],
                                     warm_sb[:], start=True, stop=True)
                    nc.vector.tensor_copy(warm_sb[0:64, :], wp[:, 0:64])
                    nc.vector.tensor_copy(warm_sb[64:128, :], wp[:, 0:64])

            def edge_phase(it):
                src_dram = feat0 if it == 1 else agouts[it - 2]
                idxt = idx0_sb if it == 1 else idx_sb
                gsz = T // GATHER_CHUNKS
                for gc in range(GATHER_CHUNKS):
                    nc.gpsimd.dma_gather(
                        g_sb[:, gc * gsz * FB:(gc + 1) * gsz * FB].rearrange(
                            "p (t o) -> p t o", o=FB
                        ),
                        src_dram[:],
                        idxt[:, gc * gsz * 8:(gc + 1) * gsz * 8],
                        gsz * EPT,
                        gsz * EPT,
                        FB,
                        queue_num=gc % N_SWDGE_QUEUES,
                    )
                # mm1: S = G^T @ (Qh + Ql) per tile; PSUM partitions 0:64 get
                # the hi-feature products, 64:128 the lo-feature products.
                t = 0
                while t < T:
                    ntl = min(3, T - t)
                    s_ps = ps.tile([128, 3 * QW], f32, tag="s", name=f"sps{it}_{t}")
                    for j in range(ntl):
                        nc.tensor.matmul(
                            s_ps[:, j * QW:(j + 1) * QW],
                            g_sb[:, (t + j) * FB:(t + j + 1) * FB],
                            qh_sb[:, (t + j) * QW:(t + j + 1) * QW],
                            start=True, stop=False,
                        )
                        nc.tensor.matmul(
                            s_ps[:, j * QW:(j + 1) * QW],
                            g_sb[:, (t + j) * FB:(t + j + 1) * FB],
                            ql_sb[:, (t + j) * QW:(t + j + 1) * QW],
                            start=False, stop=True,
                        )
                    if (t // 3) % 2 == 0:
                        nc.scalar.copy(sh_sb[:, t * QW:(t + ntl) * QW],
                                       s_ps[:, :ntl * QW])
                    else:
                        nc.vector.tensor_copy(sh_sb[:, t * QW:(t + ntl) * QW],
                                              s_ps[:, :ntl * QW])
                    # lo split only needed for the hi-feature (A) half; the
                    # lo-feature (B) rows stay zero (their lo is ~2^-18).
                    nc.vector.tensor_sub(sl_sb[0:64, t * QW:(t + ntl) * QW],
                                         s_ps[0:64, :ntl * QW],
                                         sh_sb[0:64, t * QW:(t + ntl) * QW])
                    t += ntl

            sh_re = sh_sb[:].rearrange("p (t c s) -> p t c s", c=NCH, s=SLOTS)
            sl_re = sl_sb[:].rearrange("p (t c s) -> p t c s", c=NCH, s=SLOTS)

            def dense_chunk(it, k, c0, w, h32, hs1, hs2, hn32, hs1n, hs2n):
                sl = slice(c0, c0 + w)
                t0, tn = c0 // SLOTS, w // SLOTS
                # mm2 (+root) for this chunk's 16 tiles
                agg = aggp.tile([64, CHUNK], f32, tag="agg", name=f"agg{it}_{k}")
                for c in range(NCH):
                    nc.tensor.matmul(agg[:, :w], ws_sb[:, c * 128:c * 128 + 64],
                                     sh_re[:, t0:t0 + tn, c, :],
                                     start=(c == 0), stop=False)
                    nc.tensor.matmul(agg[:, :w], ws_sb[:, c * 128:c * 128 + 64],
                                     sl_re[:, t0:t0 + tn, c, :],
                                     start=False, stop=False)
                    nc.tensor.matmul(agg[:, :w], ws_sb[:, c * 128 + 64:c * 128 + 128],
                                     sh_re[:, t0:t0 + tn, c, :],
                                     start=False, stop=False)
                nc.tensor.matmul(agg[:, :w], root_sb[:], hs1[:, sl],
                                 start=False, stop=False)
                nc.tensor.matmul(agg[:, :w], root_sb[:], hs2[:, sl],
                                 start=False, stop=True)
                # m = relu(agg + conv_b); bf16 stacks [mh;ml] and [ml;mh]
                m32 = small.tile([128, CHUNK], f32, tag="m32", name=f"m32_{it}{k}")
                nc.scalar.activation(m32[0:64, :w], agg[:, :w], AF.Relu, bias=convb)
                ms1 = small.tile([128, CHUNK], bf16, tag="ms1", name=f"ms1_{it}{k}")
                ms2 = small.tile([128, CHUNK], bf16, tag="ms2", name=f"ms2_{it}{k}")
                nc.vector.tensor_copy(ms1[0:64, :w], m32[0:64, :w])
                nc.vector.tensor_sub(ms1[64:128, :w], m32[0:64, :w], ms1[0:64, :w])
                nc.gpsimd.tensor_copy(ms2[0:64, :w], ms1[64:128, :w])
                nc.gpsimd.tensor_copy(ms2[64:128, :w], ms1[0:64, :w])

                # r|z stacked on 128 partitions: 4 FWL matmuls
                rz = grz.tile([128, CHUNK], f32, tag="rz", name=f"rz{it}_{k}")
                nc.tensor.matmul(rz[:, :w], gru_sb[:, 0:128], ms1[:, :w],
                                 start=True, stop=False)
                nc.tensor.matmul(rz[:, :w], gru_sb[:, 0:128], ms2[:, :w],
                                 start=False, stop=False)
                nc.tensor.matmul(rz[:, :w], gru_sb[:, 128:256], hs1[:, sl],
                                 start=False, stop=False)
                nc.tensor.matmul(rz[:, :w], gru_sb[:, 128:256], hs2[:, sl],
                                 start=False, stop=True)
                rz_sb = small.tile([128, CHUNK], f32, tag="rzsb", name=f"rzs{it}{k}")
                nc.scalar.activation(rz_sb[:, :w], rz[:, :w], AF.Sigmoid, bias=brz)
                n1 = gn.tile([64, CHUNK], f32, tag="gn", name=f"n1{it}_{k}")
                nc.tensor.matmul(n1[:, :w], gru_sb[:, 256:320], ms1[:, :w],
                                 start=True, stop=False)
                nc.tensor.matmul(n1[:, :w], gru_sb[:, 256:320], ms2[:, :w],
                                 start=False, stop=True)
                n2 = gn.tile([64, CHUNK], f32, tag="gn", name=f"n2{it}_{k}")
                nc.tensor.matmul(n2[:, :w], gru_sb[:, 320:384], hs1[:, sl],
                                 start=True, stop=False)
                nc.tensor.matmul(n2[:, :w], gru_sb[:, 320:384], hs2[:, sl],
                                 start=False, stop=True)
                # tmp = (n2 + b_hh_n) * r
                tmp = small.tile([64, CHUNK], f32, tag="tmp", name=f"tmp{it}{k}")
                nc.vector.scalar_tensor_tensor(
                    tmp[:, :w], n2[:, :w], bnhh, rz_sb[0:64, :w], ALU.add, ALU.mult
                )
                pre = small.tile([64, CHUNK], f32, tag="pre", name=f"pre{it}{k}")
                nc.vector.tensor_add(pre[:, :w], n1[:, :w], tmp[:, :w])
                nsb = small.tile([128, CHUNK], f32, tag="nsb", name=f"nsb{it}{k}")
                nc.scalar.activation(nsb[0:64, :w], pre[:, :w], AF.Tanh, bias=bnih)
                # h' = n + z * (h - n)
                dd = small.tile([128, CHUNK], f32, tag="dd", name=f"dd{it}{k}")
                nc.vector.tensor_sub(dd[0:64, :w], h32[0:64, sl], nsb[0:64, :w])
                t4 = small.tile([128, CHUNK], f32, tag="t4", name=f"t4{it}{k}")
                z_sb = small.tile([128, CHUNK], f32, tag="zsb", name=f"zsb{it}{k}")
                nc.scalar.copy(z_sb[0:64, :w], rz_sb[64:128, :w])
                nc.vector.tensor_mul(t4[0:64, :w], z_sb[0:64, :w], dd[0:64, :w])
                nc.vector.tensor_add(hn32[0:64, sl], nsb[0:64, :w], t4[0:64, :w])
                if it < 3:
                    # bf16 split (and swap) of the new h for the next iteration
                    nc.scalar.copy(hs1n[0:64, sl], hn32[0:64, sl])
                    nc.gpsimd.tensor_sub(hs1n[64:128, sl], hn32[0:64, sl],
                                           hs1n[0:64, sl])
                    nc.sync.dma_start(hs2n[0:64, sl], hs1n[64:128, sl])
                    nc.sync.dma_start(hs2n[64:128, sl], hs1n[0:64, sl])

            def writeback_chunk(it, k, c0, w, hs1n, hn32):
                nb = w // 128
                if it < 3:
                    tp = tpp.tile([128, 512], bf16, tag="tp", name=f"w{it}_{k}")
                    for b in range(nb):
                        bs = slice(c0 + b * 128, c0 + (b + 1) * 128)
                        nc.tensor.transpose(tp[:, b * 128:(b + 1) * 128],
                                            hs1n[:, bs], ident_sb[:])
                    if k % 2 == 0:
                        nc.scalar.copy(rowb[:, c0:c0 + w], tp[:, :nb * 128])
                    else:
                        nc.vector.tensor_copy(rowb[:, c0:c0 + w], tp[:, :nb * 128])
                    agin, agout = agins[it - 1], agouts[it - 1]
                    nc.sync.dma_start(
                        agin[:].rearrange("(t p) f -> p t f", p=128)[
                            :, c0 // 128:c0 // 128 + nb, :],
                        rowb[:, c0:c0 + w].rearrange("p (t f) -> p t f", f=FB),
                    )
                    if c0 + w == NC_COLS:
                        # Shared DRAM allows a single writer instruction, so
                        # the AllGather fires once, after the last chunk DMA.
                        nc.gpsimd.collective_compute(
                            "AllGather",
                            mybir.AluOpType.bypass,
                            replica_groups=[list(range(NCORES))],
                            ins=[agin[:]],
                            outs=[agout[:]],
                        )
                else:
                    # final iteration: write fp32 h to the external output
                    tpf = tpp.tile([128, 256], f32, tag="tp", name=f"wf_{k}")
                    for b in range(nb):
                        bs = slice(c0 + b * 128, c0 + (b + 1) * 128)
                        nc.tensor.transpose(
                            tpf[:, b * 64:(b + 1) * 64], hn32[0:64, bs],
                            identf_sb[:])
                    if k % 2 == 0:
                        nc.scalar.copy(rowf[:, (c0 // 128) * H:(c0 // 128 + nb) * H],
                                       tpf[:, :nb * 64])
                    else:
                        nc.vector.tensor_copy(
                            rowf[:, (c0 // 128) * H:(c0 // 128 + nb) * H],
                            tpf[:, :nb * 64])

            nc.gpsimd.memzero(sl_sb[64:128, :])
            nc.vector.tensor_copy(warm_sb[:], ident_sb[:, 0:64])
            h32, hn32 = h32a, h32b
            cur = (hs1a, hs2a, hs1b, hs2b)
            for it in (1, 2, 3):
                hs1, hs2, hs1n, hs2n = cur
                warm_chain(8 if it == 1 else 56, it)
                edge_phase(it)
                for k, (c0, w) in enumerate(chunks):
                    dense_chunk(it, k, c0, w, h32, hs1, hs2, hn32, hs1n, hs2n)
                    writeback_chunk(it, k, c0, w, hs1n, hn32)
                h32, hn32 = hn32, h32
                cur = (hs1n, hs2n, hs1, hs2)

            nc.sync.dma_start(
                out_ext[:].rearrange("(t p) o -> p t o", p=128),
                rowf[:].rearrange("p (t o) -> p t o", o=H),
            )

    nc.compile()
    _NC_CACHE["nc"] = nc
    return nc


# ----------------------------------------------------------------------------
# host-side graph preprocessing (pure data layout, no model FLOPs)
# ----------------------------------------------------------------------------
def _bf16_rne(x):
    """Round fp32 -> bf16 (round-to-nearest-even). Returns (f32val, uint16bits)."""
    u = np.asarray(x, np.float32).view(np.uint32)
    r = (u + 0x7FFF + ((u >> 16) & 1)) & 0xFFFF0000
    return r.view(np.float32), (r >> 16).astype(np.uint16)


def _bf16_split(x):
    """x ~= hi + lo with both bf16. Returns (hi_f32, lo_f32, hi_u16, lo_u16)."""
    x = np.ascontiguousarray(np.asarray(x, np.float32))
    hi_f, hi_u = _bf16_rne(x)
    lo_f, lo_u = _bf16_rne(x - hi_f)
    return hi_f, lo_f, hi_u, lo_u


def _pack(edge_index, edge_attr):
    src = np.asarray(edge_index[0]).astype(np.int64)
    dst = np.asarray(edge_index[1]).astype(np.int64)
    ea = np.asarray(edge_attr, np.float32)
    order = np.argsort(dst, kind="stable")
    ssrc, sea = src[order], ea[order]
    deg = np.bincount(dst, minlength=N_NODES)
    starts = np.zeros(N_NODES + 1, np.int64)
    starts[1:] = np.cumsum(deg)
    uniq = np.flatnonzero(deg)
    zs = np.flatnonzero(deg == 0)
    node_seq = np.concatenate([uniq, zs])

    raw_tiles = [[]]
    ce = 0
    for nd in node_seq:
        d = int(deg[nd])
        assert d <= EPT, f"node degree {d} exceeds edge tile capacity"
        if len(raw_tiles[-1]) >= SLOTS or ce + d > EPT:
            raw_tiles.append([])
            ce = 0
        raw_tiles[-1].append(int(nd))
        ce += d
    assert len(raw_tiles) <= NTILES, f"need {len(raw_tiles)} tiles > {NTILES}"
    # Distribute real tiles round-robin across the 8 cores so every core gets
    # an equal share of real edges (a contiguous split leaves the last core
    # nearly all padding, which skews its runtime and stalls the collectives).
    tiles_nodes = [[] for _ in range(NTILES)]
    for i, nodes in enumerate(raw_tiles):
        core, j = i % NCORES, i // NCORES
        tiles_nodes[core * T + j] = nodes

    perm = np.empty(N_NODES, np.int64)
    for t, nodes in enumerate(tiles_nodes):
        for j, nd in enumerate(nodes):
            perm[nd] = t * SLOTS + j

    q = np.zeros((NTILES, EPT, NCH, SLOTS), np.float32)
    # Padding gather slots must NOT all point at row 0: thousands of reads of
    # one 256B row serialize on a single HBM bank.  Padding gets spread
    # distinct rows in the core's own slice.
    srcslot = np.full((NTILES, EPT), -1, np.int16)
    for t, nodes in enumerate(tiles_nodes):
        e = 0
        for j, nd in enumerate(nodes):
            s0, s1 = int(starts[nd]), int(starts[nd + 1])
            ne = s1 - s0
            if ne:
                q[t, e:e + ne, 0:4, j] = sea[s0:s1]
                q[t, e:e + ne, 4, j] = 1.0
                srcslot[t, e:e + ne] = perm[ssrc[s0:s1]].astype(np.int16)
                e += ne
    for k in range(NCORES):
        base = k * NC_COLS
        block = srcslot[k * T:(k + 1) * T].reshape(-1)
        holes = np.flatnonzero(block < 0)
        block[holes] = base + np.arange(holes.size) % NC_COLS
        srcslot[k * T:(k + 1) * T] = block.reshape(T, EPT)

    _, _, qh_u, ql_u = _bf16_split(q)
    qhs, qls, idxs, idx0s, srclists = [], [], [], [], []
    i_arange = np.arange(T * EPT)

    def wrap16(flat):
        # the index list is read per 16-partition group by each of the 8
        # GPSIMD cores on HW -> replicate it into every group
        ia = np.zeros((128, T * 8), np.int16)
        for g in range(8):
            ia[g * 16 + i_arange % 16, i_arange // 16] = flat
        return ia

    for k in range(NCORES):
        def qlay(qu):
            qt = qu[k * T:(k + 1) * T]
            return np.ascontiguousarray(qt.transpose(1, 0, 2, 3)).reshape(
                128, T * QW)
        qhs.append(qlay(qh_u.reshape(NTILES, EPT, NCH, SLOTS)))
        qls.append(qlay(ql_u.reshape(NTILES, EPT, NCH, SLOTS)))
        flat = srcslot[k * T:(k + 1) * T].reshape(-1).astype(np.int64)
        idxs.append(wrap16(flat.astype(np.int16)))
        # compact source space for iteration 1: only the rows this core
        # actually gathers exist in its local feat0 table
        srcs, inv = np.unique(flat, return_inverse=True)
        assert srcs.size <= NSRC0, f"core {k}: {srcs.size} > {NSRC0}"
        srclists.append(srcs)
        idx0s.append(wrap16(inv.astype(np.int16)))
    return qhs, qls, idxs, idx0s, srclists, perm


def _prep_inputs(inputs):
    x = np.asarray(inputs["x"], np.float32)
    qhs, qls, idxs, idx0s, srclists, perm = _pack(
        inputs["edge_index"], inputs["edge_attr"])

    x_pad = np.zeros((NPAD, IN_F), np.float32)
    x_pad[perm] = x
    xt = np.ascontiguousarray(x_pad.T)                      # [32, NPAD]
    _, _, xh_u, xl_u = _bf16_split(xt)
    xs_full = np.concatenate([xh_u, xl_u], axis=0)          # [64, NPAD] u16
    xsos = [
        np.ascontiguousarray(xs_full[:, k * NC_COLS:(k + 1) * NC_COLS])
        for k in range(NCORES)
    ]
    xs0s = []
    for k in range(NCORES):
        xs0 = np.zeros((64, NSRC0), np.uint16)
        xs0[:, :srclists[k].size] = xs_full[:, srclists[k]]
        xs0s.append(xs0)

    def stack2(a):                                          # [m,n] -> [2m,n]
        return np.concatenate([a, a], axis=0)

    lin0_w = np.asarray(inputs["lin0_w"], np.float32)       # [32, 64]
    _, _, w0h, w0l = _bf16_split(lin0_w)
    l0 = np.zeros((64, 128), np.uint16)
    l0[:, 0:64] = stack2(w0h)
    l0[:, 64:128] = stack2(w0l)

    nw = np.asarray(inputs["nn_w"], np.float32)
    ws = np.zeros((128, NCH * 128), np.uint16)
    for c in range(NCH):
        wc = (nw[c].reshape(H, H) if c < 4
              else np.asarray(inputs["nn_b"], np.float32).reshape(H, H))
        _, _, wch, wcl = _bf16_split(wc)
        ws[:, c * 128:c * 128 + 64] = stack2(wch)
        ws[:, c * 128 + 64:c * 128 + 128] = stack2(wcl)

    root_w = np.asarray(inputs["root_w"], np.float32)
    _, _, rh, rl = _bf16_split(root_w)
    root = np.concatenate([rh, rl], axis=0)                 # [128, 64]

    wih_t = np.ascontiguousarray(np.asarray(inputs["gru_w_ih"], np.float32).T)
    whh_t = np.ascontiguousarray(np.asarray(inputs["gru_w_hh"], np.float32).T)
    _, _, wih_h, wih_l = _bf16_split(wih_t)                 # [64, 192]
    _, _, whh_h, whh_l = _bf16_split(whh_t)
    gru = np.zeros((128, 384), np.uint16)
    gru[0:64, 0:128] = wih_h[:, 0:128]                      # Vih_rz
    gru[64:128, 0:128] = wih_l[:, 0:128]
    gru[0:64, 128:256] = whh_h[:, 0:128]                    # Vhh_rz
    gru[64:128, 128:256] = whh_l[:, 0:128]
    gru[0:64, 256:320] = wih_h[:, 128:192]                  # Vih_n
    gru[64:128, 256:320] = wih_l[:, 128:192]
    gru[0:64, 320:384] = whh_h[:, 128:192]                  # Vhh_n
    gru[64:128, 320:384] = whh_l[:, 128:192]

    b_ih = np.asarray(inputs["gru_b_ih"], np.float32)
    b_hh = np.asarray(inputs["gru_b_hh"], np.float32)
    bias_pack = np.zeros((128, 8), np.float32)
    bias_pack[0:64, 0] = np.asarray(inputs["lin0_b"], np.float32)
    bias_pack[0:64, 1] = np.asarray(inputs["conv_b"], np.float32)
    bias_pack[0:64, 2] = (b_ih + b_hh)[0:64]                # b_r
    bias_pack[64:128, 2] = (b_ih + b_hh)[64:128]            # b_z
    bias_pack[0:64, 4] = b_ih[128:192]
    bias_pack[0:64, 5] = b_hh[128:192]
    identf = np.eye(64, dtype=np.float32)
    _, ident_u = _bf16_rne(np.eye(128, dtype=np.float32))

    in_maps = []
    for k in range(NCORES):
        in_maps.append(
            {
                "qh_in": qhs[k],
                "ql_in": qls[k],
                "idx_in": idxs[k],
                "idx0_in": idx0s[k],
                "xs0_in": xs0s[k],
                "xso_in": xsos[k],
                "l0_in": l0,
                "ws_in": ws,
                "root_in": root,
                "gru_in": gru,
                "bias_in": bias_pack,
                "ident_in": ident_u,
                "identf_in": identf,
            }
        )
    return in_maps, perm


def _assemble(results, perm):
    full = np.concatenate([results[k]["out_sl"] for k in range(NCORES)], axis=0)
    return np.ascontiguousarray(full[perm]).astype(np.float32)


def kernel(**inputs) -> np.ndarray:
    in_maps, perm = _prep_inputs(inputs)
    nc = _get_nc()
    if os.environ.get("BASS_KERNEL_SIM"):
        results = _run_sim(nc, in_maps)
    else:
        from concourse import bass_utils

        res = bass_utils.run_bass_kernel_spmd(
            nc, in_maps, core_ids=list(range(NCORES))
        )
        results = res.results
    return _assemble(results, perm)


def _run_sim(nc, in_maps):
    from concourse.bass_interp import MultiCoreSim

    sim = MultiCoreSim(nc, num_cores=NCORES, trace=False)
    for k, core in sim.cores.items():
        for name, arr in in_maps[k].items():
            core.tensor(name)[:] = arr
    sim.simulate(check_with_hw=False)
    out = []
    for k in range(NCORES):
        out.append({"out_sl": np.array(sim.cores[k].tensor("out_sl"))})
    return out


if __name__ == "__main__":
    rng = np.random.default_rng(0)
    demo = {
        "x": rng.standard_normal((N_NODES, IN_F), dtype=np.float32),
        "edge_index": rng.integers(0, N_NODES, (2, N_EDGES)).astype(np.int32),
        "edge_attr": rng.random((N_EDGES, 4), dtype=np.float32),
        "lin0_w": rng.standard_normal((IN_F, H), dtype=np.float32) * 0.1,
        "lin0_b": np.zeros(H, np.float32),
        "nn_w": rng.standard_normal((4, H * H), dtype=np.float32) * 0.05,
        "nn_b": np.zeros(H * H, np.float32),
        "root_w": rng.standard_normal((H, H), dtype=np.float32) * 0.1,
        "conv_b": np.zeros(H, np.float32),
        "gru_w_ih": rng.standard_normal((3 * H, H), dtype=np.float32) * 0.1,
        "gru_w_hh": rng.standard_normal((3 * H, H), dtype=np.float32) * 0.1,
        "gru_b_ih": np.zeros(3 * H, np.float32),
        "gru_b_hh": np.zeros(3 * H, np.float32),
    }
    out = kernel(**demo)
    print("kernel output", out.shape, out.dtype, float(np.abs(out).mean()))


# revision 28
# speedup vs baseline: 1.1215x; 1.1215x over previous
"""Bass/Trainium2 kernel for nn_Net_19602230739296 (NNConv + GRU message passing GNN).

Algorithm (mathematically equivalent to the reference):
  theta[e] = (edge_attr[e] @ nn_w + nn_b).reshape(H, H) is never materialized.
  msg[e]   = sum_c ea'[e,c] * (out[src_e] @ W_c)   with ea' = [edge_attr, 1],
             W_c = nn_w[c].reshape(H,H) for c<4, W_4 = nn_b.reshape(H,H).
  agg^T    = sum_c W_c^T @ (G^T @ Q_c)  per 128-edge tile, where G = out[src]
             (gathered rows) and Q_c[e, slot] = ea'[e,c] * [dst_e == slot-node]
             is a host-precomputed weighted one-hot "scatter" matrix.

Numerics: every fp32 value on the matmul paths is represented as a bf16
hi/lo pair (hi = bf16(x), lo = bf16(x - hi)).  bf16 matmuls run at 1 PE
cycle/row vs fp32's 4, and the PE multiplies bf16 exactly with fp32
accumulation, so a 3-term product (hi*hi + hi*lo + lo*hi) is accurate to
~2^-18 relative -- far inside the 2e-2 harness gate.  Node features live in
DRAM as [node, 128] rows = (hi 64 | lo 64) bf16, so one 256B-row gather
feeds the edge matmul with both terms and the per-tile matmul computes the
hi- and lo- partial products in one pass (128-partition PSUM output).

Sharding: edges are sorted by destination and packed into tiles of <=128
edges covering <=32 whole destination nodes.  Real tiles are dealt
round-robin across the 8 cores so each core gets an equal share of edges.
Nodes are renumbered to (tile*32 + slot).  A core's edges land only in its
own node range, so no cross-core reduction is needed.  The evolving node
features are replicated via AllGather each iteration (chunked, so the
collective overlaps the tail of the GRU); iteration 0's features are
computed for ALL nodes on every core (lin0 is tiny), which removes one
AllGather entirely.
"""
import os
import sys

import numpy as np


def _ensure_path():
    for p in ("/opt/trn_rl_repo", os.path.expanduser("~/.axon_site/_ro/trn_rl_repo")):
        if os.path.isdir(p) and p not in sys.path:
            sys.path.insert(0, p)
    try:
        import concourse  # noqa: F401
    except ImportError as e:  # pragma: no cover
        raise ImportError(f"concourse (bass) not importable: {e}")


_ensure_path()

N_NODES, N_EDGES, IN_F, H = 10000, 50000, 32, 64
NCORES = 8
SLOTS = 24            # destination-node slots per tile
EPT = 128             # edge slots per tile
NCH = 5               # edge_attr channels (4) + constant channel for nn_b
T = 64                # tiles per core (fixed so the compiled NEFF is shape-stable)
NTILES = NCORES * T   # 448
NC_COLS = T * SLOTS   # padded nodes per core (1792)
NPAD = NCORES * NC_COLS
CHUNK = 384
GATHER_CHUNKS = 16
N_SWDGE_QUEUES = 4
QW = NCH * SLOTS      # Q columns per tile
NSRC0 = 6144          # compact it-0 feature-table rows (unique sources, padded)
FB = 2 * H            # 128 bf16 feature bytes-row: hi|lo


def _chunks():
    out = []
    c0 = 0
    while c0 < NC_COLS:
        w = min(CHUNK, NC_COLS - c0)
        out.append((c0, w))
        c0 += w
    return out


# ----------------------------------------------------------------------------
# device program
# ----------------------------------------------------------------------------
_NC_CACHE = {}


def _get_nc():
    if "nc" in _NC_CACHE:
        return _NC_CACHE["nc"]
    import concourse.bacc as bacc
    import concourse.mybir as mybir
    import concourse.tile as tile

    dt = mybir.dt
    f32, i16, bf16 = dt.float32, dt.int16, dt.bfloat16
    AF = mybir.ActivationFunctionType
    ALU = mybir.AluOpType

    nc = bacc.Bacc(
        "TRN2",
        target_bir_lowering=False,
        debug=False,
        enable_asserts=False,
        num_devices=NCORES,
        num_swdge_queues=N_SWDGE_QUEUES,
    )

    qh_in = nc.dram_tensor("qh_in", [128, T * QW], bf16, kind="ExternalInput").ap()
    ql_in = nc.dram_tensor("ql_in", [128, T * QW], bf16, kind="ExternalInput").ap()
    idx_in = nc.dram_tensor("idx_in", [128, T * 8], i16, kind="ExternalInput").ap()
    xs0_in = nc.dram_tensor("xs0_in", [64, NSRC0], bf16, kind="ExternalInput").ap()
    idx0_in = nc.dram_tensor("idx0_in", [128, T * 8], i16, kind="ExternalInput").ap()
    xso_in = nc.dram_tensor("xso_in", [64, NC_COLS], bf16, kind="ExternalInput").ap()
    l0_in = nc.dram_tensor("l0_in", [64, 128], bf16, kind="ExternalInput").ap()
    ws_in = nc.dram_tensor("ws_in", [128, NCH * 128], bf16, kind="ExternalInput").ap()
    root_in = nc.dram_tensor("root_in", [128, 64], bf16, kind="ExternalInput").ap()
    gru_in = nc.dram_tensor("gru_in", [128, 384], bf16, kind="ExternalInput").ap()
    bias_in = nc.dram_tensor("bias_in", [128, 8], f32, kind="ExternalInput").ap()
    ident_in = nc.dram_tensor("ident_in", [128, 128], bf16, kind="ExternalInput").ap()
    identf_in = nc.dram_tensor("identf_in", [64, 64], f32, kind="ExternalInput").ap()
    out_ext = nc.dram_tensor("out_sl", [NC_COLS, H], f32, kind="ExternalOutput").ap()

    chunks = _chunks()

    with tile.TileContext(nc) as tc:
        with tc.tile_pool(name="const", bufs=1) as const, \
             tc.tile_pool(name="work", bufs=1) as work, \
             tc.tile_pool(name="small", bufs=2) as small, \
             tc.tile_pool(name="ps", bufs=2, space="PSUM") as ps, \
             tc.tile_pool(name="aggp", bufs=2, space="PSUM") as aggp, \
             tc.tile_pool(name="grz", bufs=1, space="PSUM") as grz, \
             tc.tile_pool(name="gn", bufs=2, space="PSUM") as gn, \
             tc.tile_pool(name="tpp", bufs=1, space="PSUM") as tpp, \
             tc.tile_pool(name="dram", bufs=1, space="DRAM") as dram:

            qh_sb = const.tile([128, T * QW], bf16, name="qh_sb")
            ql_sb = const.tile([128, T * QW], bf16, name="ql_sb")
            idx_sb = const.tile([128, T * 8], i16, name="idx_sb")
            xs0_sb = const.tile([64, NSRC0], bf16, name="xs0_sb")
            idx0_sb = const.tile([128, T * 8], i16, name="idx0_sb")
            xso_sb = const.tile([64, NC_COLS], bf16, name="xso_sb")
            l0_sb = const.tile([64, 128], bf16, name="l0_sb")
            ws_sb = const.tile([128, NCH * 128], bf16, name="ws_sb")
            root_sb = const.tile([128, 64], bf16, name="root_sb")
            gru_sb = const.tile([128, 384], bf16, name="gru_sb")
            bias_sb = const.tile([128, 8], f32, name="bias_sb")
            ident_sb = const.tile([128, 128], bf16, name="ident_sb")
            identf_sb = const.tile([64, 64], f32, name="identf_sb")

            for sb_t, in_t in (
                (xso_sb, xso_in), (l0_sb, l0_in), (bias_sb, bias_in),
                (ident_sb, ident_in), (identf_sb, identf_in),
                (xs0_sb, xs0_in), (idx0_sb, idx0_in), (idx_sb, idx_in),
                (gru_sb, gru_in), (root_sb, root_in), (ws_sb, ws_in),
                (qh_sb, qh_in), (ql_sb, ql_in),
            ):
                nc.sync.dma_start(sb_t[:], in_t[:])

            l0b = bias_sb[0:64, 0:1]
            convb = bias_sb[0:64, 1:2]
            brz = bias_sb[:, 2:3]          # [br ; bz] stacked on 128 partitions
            bnih = bias_sb[0:64, 4:5]
            bnhh = bias_sb[0:64, 5:6]

            h32a = work.tile([128, NC_COLS], f32, name="h32a")
            h32b = work.tile([128, NC_COLS], f32, name="h32b")
            # h bf16 split, stacked [hi;lo] and swapped [lo;hi] on 128 parts
            hs1a = work.tile([128, NC_COLS], bf16, name="hs1a")
            hs2a = work.tile([128, NC_COLS], bf16, name="hs2a")
            hs1b = work.tile([128, NC_COLS], bf16, name="hs1b")
            hs2b = work.tile([128, NC_COLS], bf16, name="hs2b")
            g_sb = work.tile([128, T * FB], bf16, name="g_sb")
            sh_sb = work.tile([128, T * QW], bf16, name="sh_sb")
            sl_sb = work.tile([128, T * QW], bf16, name="sl_sb")
            rowb = work.tile([128, NC_COLS], bf16, name="rowb")
            rowf = work.tile([128, (NC_COLS // 128) * H], f32, name="rowf")
            warm_sb = work.tile([128, 64], bf16, name="warm_sb")

            feat0 = dram.tile([NSRC0, FB], bf16, name="feat0")
            agins = [dram.tile([NC_COLS, FB], bf16, name=f"agin{i}") for i in (1, 2)]
            agouts = [
                dram.tile([NPAD, FB], bf16, addr_space="Shared", name=f"agout{i}")
                for i in (1, 2)
            ]

            # ---- iteration 0: out0 = relu(x @ lin0_w + lin0_b) --------------
            # Full-table pass: every core computes out0 for ALL nodes and
            # writes the bf16 hi|lo feature table locally -- no collective.
            for c in range(NSRC0 // CHUNK):
                sl = slice(c * CHUNK, (c + 1) * CHUNK)
                p0 = gn.tile([64, CHUNK], f32, tag="gn", name=f"l0f{c}")
                nc.tensor.matmul(p0[:], l0_sb[:, 0:64], xs0_sb[:, sl],
                                 start=True, stop=False)
                nc.tensor.matmul(p0[:], l0_sb[:, 64:128], xs0_sb[:, sl],
                                 start=False, stop=True)
                o32 = small.tile([128, CHUNK], f32, tag="o32", name=f"o32_{c}")
                nc.scalar.activation(o32[0:64, :], p0[:], AF.Relu, bias=l0b)
                osk = small.tile([128, CHUNK], bf16, tag="osk", name=f"osk_{c}")
                nc.vector.tensor_copy(osk[0:64, :], o32[0:64, :])
                nc.vector.tensor_sub(osk[64:128, :], o32[0:64, :], osk[0:64, :])
                tp = tpp.tile([128, 512], bf16, tag="tp", name=f"t0_{c}")
                for b in range(CHUNK // 128):
                    bs = slice(b * 128, (b + 1) * 128)
                    nc.tensor.transpose(tp[:, b * 128:(b + 1) * 128],
                                        osk[:, bs], ident_sb[:])
                nbc = CHUNK // 128
                rf = small.tile([128, CHUNK], bf16, tag="rf", name=f"rf_{c}")
                if c % 2 == 0:
                    nc.scalar.copy(rf[:], tp[:, :CHUNK])
                else:
                    nc.vector.tensor_copy(rf[:], tp[:, :CHUNK])
                nc.sync.dma_start(
                    feat0[:].rearrange("(t p) f -> p t f", p=128)[
                        :, nbc * c:nbc * c + nbc, :],
                    rf[:].rearrange("p (t f) -> p t f", f=FB),
                )
            # Own-slice pass (fp32 h and its bf16 split for root/GRU inputs).
            for k, (c0, w) in enumerate(chunks):
                sl = slice(c0, c0 + w)
                p1 = gn.tile([64, CHUNK], f32, tag="gn", name=f"l0o{k}")
                nc.tensor.matmul(p1[:, :w], l0_sb[:, 0:64], xso_sb[:, sl],
                                 start=True, stop=False)
                nc.tensor.matmul(p1[:, :w], l0_sb[:, 64:128], xso_sb[:, sl],
                                 start=False, stop=True)
                nc.scalar.activation(h32a[0:64, sl], p1[:, :w], AF.Relu, bias=l0b)
                nc.vector.tensor_copy(hs1a[0:64, sl], h32a[0:64, sl])
                nc.vector.tensor_sub(hs1a[64:128, sl], h32a[0:64, sl],
                                     hs1a[0:64, sl])
                nc.scalar.copy(hs2a[0:64, sl], hs1a[64:128, sl])
                nc.scalar.copy(hs2a[64:128, sl], hs1a[0:64, sl])

            def warm_chain(links, it):
                # Keep the PE's HAM activity monitor from dropping to the
                # cold 1.2GHz clock while the engine waits on the collective:
                # a dependency chain of tiny matmuls paces ~1 PE op per µs.
                for i in range(links):
                    wp = gn.tile([64, CHUNK], f32, tag="gn", name=f"wm{it}_{i}")
                    nc.tensor.matmul(wp[:, 0:64], ident_sb[:, 0:64],
                                     warm_sb[:], start=True, stop=True)
                    nc.vector.tensor_copy(warm_sb[0:64, :], wp[:, 0:64])
                    nc.vector.tensor_copy(warm_sb[64:128, :], wp[:, 0:64])

            def edge_phase(it):
                src_dram = feat0 if it == 1 else agouts[it - 2]
                idxt = idx0_sb if it == 1 else idx_sb
                gsz = T // GATHER_CHUNKS
                for gc in range(GATHER_CHUNKS):
                    nc.gpsimd.dma_gather(
                        g_sb[:, gc * gsz * FB:(gc + 1) * gsz * FB].rearrange(
                            "p (t o) -> p t o", o=FB
                        ),
                        src_dram[:],
                        idxt[:, gc * gsz * 8:(gc + 1) * gsz * 8],
                        gsz * EPT,
                        gsz * EPT,
                        FB,
                        queue_num=gc % N_SWDGE_QUEUES,
                    )
                # mm1: S = G^T @ (Qh + Ql) per tile; PSUM partitions 0:64 get
                # the hi-feature products, 64:128 the lo-feature products.
                t = 0
                while t < T:
                    ntl = min(3, T - t)
                    s_ps = ps.tile([128, 3 * QW], f32, tag="s", name=f"sps{it}_{t}")
                    for j in range(ntl):
                        nc.tensor.matmul(
                            s_ps[:, j * QW:(j + 1) * QW],
                            g_sb[:, (t + j) * FB:(t + j + 1) * FB],
                            qh_sb[:, (t + j) * QW:(t + j + 1) * QW],
                            start=True, stop=False,
                        )
                        nc.tensor.matmul(
                            s_ps[:, j * QW:(j + 1) * QW],
                            g_sb[:, (t + j) * FB:(t + j + 1) * FB],
                            ql_sb[:, (t + j) * QW:(t + j + 1) * QW],
                            start=False, stop=True,
                        )
                    nc.scalar.copy(sh_sb[:, t * QW:(t + ntl) * QW],
                                   s_ps[:, :ntl * QW])
                    nc.vector.tensor_sub(sl_sb[:, t * QW:(t + ntl) * QW],
                                         s_ps[:, :ntl * QW],
                                         sh_sb[:, t * QW:(t + ntl) * QW])
                    t += ntl

            sh_re = sh_sb[:].rearrange("p (t c s) -> p t c s", c=NCH, s=SLOTS)
            sl_re = sl_sb[:].rearrange("p (t c s) -> p t c s", c=NCH, s=SLOTS)

            def dense_chunk(it, k, c0, w, h32, hs1, hs2, hn32, hs1n, hs2n):
                sl = slice(c0, c0 + w)
                t0, tn = c0 // SLOTS, w // SLOTS
                # mm2 (+root) for this chunk's 16 tiles
                agg = aggp.tile([64, CHUNK], f32, tag="agg", name=f"agg{it}_{k}")
                for c in range(NCH):
                    nc.tensor.matmul(agg[:, :w], ws_sb[:, c * 128:c * 128 + 64],
                                     sh_re[:, t0:t0 + tn, c, :],
                                     start=(c == 0), stop=False)
                    nc.tensor.matmul(agg[:, :w], ws_sb[:, c * 128:c * 128 + 64],
                                     sl_re[:, t0:t0 + tn, c, :],
                                     start=False, stop=False)
                    nc.tensor.matmul(agg[:, :w], ws_sb[:, c * 128 + 64:c * 128 + 128],
                                     sh_re[:, t0:t0 + tn, c, :],
                                     start=False, stop=False)
                nc.tensor.matmul(agg[:, :w], root_sb[:], hs1[:, sl],
                                 start=False, stop=False)
                nc.tensor.matmul(agg[:, :w], root_sb[:], hs2[:, sl],
                                 start=False, stop=True)
                # m = relu(agg + conv_b); bf16 stacks [mh;ml] and [ml;mh]
                m32 = small.tile([128, CHUNK], f32, tag="m32", name=f"m32_{it}{k}")
                nc.scalar.activation(m32[0:64, :w], agg[:, :w], AF.Relu, bias=convb)
                ms1 = small.tile([128, CHUNK], bf16, tag="ms1", name=f"ms1_{it}{k}")
                ms2 = small.tile([128, CHUNK], bf16, tag="ms2", name=f"ms2_{it}{k}")
                nc.vector.tensor_copy(ms1[0:64, :w], m32[0:64, :w])
                nc.vector.tensor_sub(ms1[64:128, :w], m32[0:64, :w], ms1[0:64, :w])
                nc.scalar.copy(ms2[0:64, :w], ms1[64:128, :w])
                nc.scalar.copy(ms2[64:128, :w], ms1[0:64, :w])

                # r|z stacked on 128 partitions: 4 FWL matmuls
                rz = grz.tile([128, CHUNK], f32, tag="rz", name=f"rz{it}_{k}")
                nc.tensor.matmul(rz[:, :w], gru_sb[:, 0:128], ms1[:, :w],
                                 start=True, stop=False)
                nc.tensor.matmul(rz[:, :w], gru_sb[:, 0:128], ms2[:, :w],
                                 start=False, stop=False)
                nc.tensor.matmul(rz[:, :w], gru_sb[:, 128:256], hs1[:, sl],
                                 start=False, stop=False)
                nc.tensor.matmul(rz[:, :w], gru_sb[:, 128:256], hs2[:, sl],
                                 start=False, stop=True)
                rz_sb = small.tile([128, CHUNK], f32, tag="rzsb", name=f"rzs{it}{k}")
                nc.scalar.activation(rz_sb[:, :w], rz[:, :w], AF.Sigmoid, bias=brz)
                n1 = gn.tile([64, CHUNK], f32, tag="gn", name=f"n1{it}_{k}")
                nc.tensor.matmul(n1[:, :w], gru_sb[:, 256:320], ms1[:, :w],
                                 start=True, stop=False)
                nc.tensor.matmul(n1[:, :w], gru_sb[:, 256:320], ms2[:, :w],
                                 start=False, stop=True)
                n2 = gn.tile([64, CHUNK], f32, tag="gn", name=f"n2{it}_{k}")
                nc.tensor.matmul(n2[:, :w], gru_sb[:, 320:384], hs1[:, sl],
                                 start=True, stop=False)
                nc.tensor.matmul(n2[:, :w], gru_sb[:, 320:384], hs2[:, sl],
                                 start=False, stop=True)
                # tmp = (n2 + b_hh_n) * r
                tmp = small.tile([64, CHUNK], f32, tag="tmp", name=f"tmp{it}{k}")
                nc.vector.scalar_tensor_tensor(
                    tmp[:, :w], n2[:, :w], bnhh, rz_sb[0:64, :w], ALU.add, ALU.mult
                )
                pre = small.tile([64, CHUNK], f32, tag="pre", name=f"pre{it}{k}")
                nc.vector.tensor_add(pre[:, :w], n1[:, :w], tmp[:, :w])
                nsb = small.tile([128, CHUNK], f32, tag="nsb", name=f"nsb{it}{k}")
                nc.scalar.activation(nsb[0:64, :w], pre[:, :w], AF.Tanh, bias=bnih)
                # h' = n + z * (h - n)
                dd = small.tile([128, CHUNK], f32, tag="dd", name=f"dd{it}{k}")
                nc.vector.tensor_sub(dd[0:64, :w], h32[0:64, sl], nsb[0:64, :w])
                t4 = small.tile([128, CHUNK], f32, tag="t4", name=f"t4{it}{k}")
                z_sb = small.tile([128, CHUNK], f32, tag="zsb", name=f"zsb{it}{k}")
                nc.scalar.copy(z_sb[0:64, :w], rz_sb[64:128, :w])
                nc.vector.tensor_mul(t4[0:64, :w], z_sb[0:64, :w], dd[0:64, :w])
                nc.vector.tensor_add(hn32[0:64, sl], nsb[0:64, :w], t4[0:64, :w])
                if it < 3:
                    # bf16 split (and swap) of the new h for the next iteration
                    nc.scalar.copy(hs1n[0:64, sl], hn32[0:64, sl])
                    nc.vector.tensor_sub(hs1n[64:128, sl], hn32[0:64, sl],
                                         hs1n[0:64, sl])
                    nc.sync.dma_start(hs2n[0:64, sl], hs1n[64:128, sl])
                    nc.sync.dma_start(hs2n[64:128, sl], hs1n[0:64, sl])

            def writeback_chunk(it, k, c0, w, hs1n, hn32):
                nb = w // 128
                if it < 3:
                    tp = tpp.tile([128, 512], bf16, tag="tp", name=f"w{it}_{k}")
                    for b in range(nb):
                        bs = slice(c0 + b * 128, c0 + (b + 1) * 128)
                        nc.tensor.transpose(tp[:, b * 128:(b + 1) * 128],
                                            hs1n[:, bs], ident_sb[:])
                    if k % 2 == 0:
                        nc.scalar.copy(rowb[:, c0:c0 + w], tp[:, :nb * 128])
                    else:
                        nc.vector.tensor_copy(rowb[:, c0:c0 + w], tp[:, :nb * 128])
                    agin, agout = agins[it - 1], agouts[it - 1]
                    nc.sync.dma_start(
                        agin[:].rearrange("(t p) f -> p t f", p=128)[
                            :, c0 // 128:c0 // 128 + nb, :],
                        rowb[:, c0:c0 + w].rearrange("p (t f) -> p t f", f=FB),
                    )
                    if c0 + w == NC_COLS:
                        # Shared DRAM allows a single writer instruction, so
                        # the AllGather fires once, after the last chunk DMA.
                        nc.gpsimd.collective_compute(
                            "AllGather",
                            mybir.AluOpType.bypass,
                            replica_groups=[list(range(NCORES))],
                            ins=[agin[:]],
                            outs=[agout[:]],
                        )
                else:
                    # final iteration: write fp32 h to the external output
                    tpf = tpp.tile([128, 256], f32, tag="tp", name=f"wf_{k}")
                    for b in range(nb):
                        bs = slice(c0 + b * 128, c0 + (b + 1) * 128)
                        nc.tensor.transpose(
                            tpf[:, b * 64:(b + 1) * 64], hn32[0:64, bs],
                            identf_sb[:])
                    if k % 2 == 0:
                        nc.scalar.copy(rowf[:, (c0 // 128) * H:(c0 // 128 + nb) * H],
                                       tpf[:, :nb * 64])
                    else:
                        nc.vector.tensor_copy(
                            rowf[:, (c0 // 128) * H:(c0 // 128 + nb) * H],
                            tpf[:, :nb * 64])

            nc.vector.tensor_copy(warm_sb[:], ident_sb[:, 0:64])
            h32, hn32 = h32a, h32b
            cur = (hs1a, hs2a, hs1b, hs2b)
            for it in (1, 2, 3):
                hs1, hs2, hs1n, hs2n = cur
                warm_chain(8 if it == 1 else 56, it)
                edge_phase(it)
                for k, (c0, w) in enumerate(chunks):
                    dense_chunk(it, k, c0, w, h32, hs1, hs2, hn32, hs1n, hs2n)
                    writeback_chunk(it, k, c0, w, hs1n, hn32)
                h32, hn32 = hn32, h32
                cur = (hs1n, hs2n, hs1, hs2)

            nc.sync.dma_start(
                out_ext[:].rearrange("(t p) o -> p t o", p=128),
                rowf[:].rearrange("p (t o) -> p t o", o=H),
            )

    nc.compile()
    _NC_CACHE["nc"] = nc
    return nc


# ----------------------------------------------------------------------------
# host-side graph preprocessing (pure data layout, no model FLOPs)
# ----------------------------------------------------------------------------
def _bf16_rne(x):
    """Round fp32 -> bf16 (round-to-nearest-even). Returns (f32val, uint16bits)."""
    u = np.asarray(x, np.float32).view(np.uint32)
    r = (u + 0x7FFF + ((u >> 16) & 1)) & 0xFFFF0000
    return r.view(np.float32), (r >> 16).astype(np.uint16)


def _bf16_split(x):
    """x ~= hi + lo with both bf16. Returns (hi_f32, lo_f32, hi_u16, lo_u16)."""
    x = np.ascontiguousarray(np.asarray(x, np.float32))
    hi_f, hi_u = _bf16_rne(x)
    lo_f, lo_u = _bf16_rne(x - hi_f)
    return hi_f, lo_f, hi_u, lo_u


def _pack(edge_index, edge_attr):
    src = np.asarray(edge_index[0]).astype(np.int64)
    dst = np.asarray(edge_index[1]).astype(np.int64)
    ea = np.asarray(edge_attr, np.float32)
    order = np.argsort(dst, kind="stable")
    ssrc, sea = src[order], ea[order]
    deg = np.bincount(dst, minlength=N_NODES)
    starts = np.zeros(N_NODES + 1, np.int64)
    starts[1:] = np.cumsum(deg)
    uniq = np.flatnonzero(deg)
    zs = np.flatnonzero(deg == 0)
    node_seq = np.concatenate([uniq, zs])

    raw_tiles = [[]]
    ce = 0
    for nd in node_seq:
        d = int(deg[nd])
        assert d <= EPT, f"node degree {d} exceeds edge tile capacity"
        if len(raw_tiles[-1]) >= SLOTS or ce + d > EPT:
            raw_tiles.append([])
            ce = 0
        raw_tiles[-1].append(int(nd))
        ce += d
    assert len(raw_tiles) <= NTILES, f"need {len(raw_tiles)} tiles > {NTILES}"
    # Distribute real tiles round-robin across the 8 cores so every core gets
    # an equal share of real edges (a contiguous split leaves the last core
    # nearly all padding, which skews its runtime and stalls the collectives).
    tiles_nodes = [[] for _ in range(NTILES)]
    for i, nodes in enumerate(raw_tiles):
        core, j = i % NCORES, i // NCORES
        tiles_nodes[core * T + j] = nodes

    perm = np.empty(N_NODES, np.int64)
    for t, nodes in enumerate(tiles_nodes):
        for j, nd in enumerate(nodes):
            perm[nd] = t * SLOTS + j

    q = np.zeros((NTILES, EPT, NCH, SLOTS), np.float32)
    # Padding gather slots must NOT all point at row 0: thousands of reads of
    # one 256B row serialize on a single HBM bank.  Padding gets spread
    # distinct rows in the core's own slice.
    srcslot = np.full((NTILES, EPT), -1, np.int16)
    for t, nodes in enumerate(tiles_nodes):
        e = 0
        for j, nd in enumerate(nodes):
            s0, s1 = int(starts[nd]), int(starts[nd + 1])
            ne = s1 - s0
            if ne:
                q[t, e:e + ne, 0:4, j] = sea[s0:s1]
                q[t, e:e + ne, 4, j] = 1.0
                srcslot[t, e:e + ne] = perm[ssrc[s0:s1]].astype(np.int16)
                e += ne
    for k in range(NCORES):
        base = k * NC_COLS
        block = srcslot[k * T:(k + 1) * T].reshape(-1)
        holes = np.flatnonzero(block < 0)
        block[holes] = base + np.arange(holes.size) % NC_COLS
        srcslot[k * T:(k + 1) * T] = block.reshape(T, EPT)

    _, _, qh_u, ql_u = _bf16_split(q)
    qhs, qls, idxs, idx0s, srclists = [], [], [], [], []
    i_arange = np.arange(T * EPT)

    def wrap16(flat):
        # the index list is read per 16-partition group by each of the 8
        # GPSIMD cores on HW -> replicate it into every group
        ia = np.zeros((128, T * 8), np.int16)
        for g in range(8):
            ia[g * 16 + i_arange % 16, i_arange // 16] = flat
        return ia

    for k in range(NCORES):
        def qlay(qu):
            qt = qu[k * T:(k + 1) * T]
            return np.ascontiguousarray(qt.transpose(1, 0, 2, 3)).reshape(
                128, T * QW)
        qhs.append(qlay(qh_u.reshape(NTILES, EPT, NCH, SLOTS)))
        qls.append(qlay(ql_u.reshape(NTILES, EPT, NCH, SLOTS)))
        flat = srcslot[k * T:(k + 1) * T].reshape(-1).astype(np.int64)
        idxs.append(wrap16(flat.astype(np.int16)))
        # compact source space for iteration 1: only the rows this core
        # actually gathers exist in its local feat0 table
        srcs, inv = np.unique(flat, return_inverse=True)
        assert srcs.size <= NSRC0, f"core {k}: {srcs.size} > {NSRC0}"
        srclists.append(srcs)
        idx0s.append(wrap16(inv.astype(np.int16)))
    return qhs, qls, idxs, idx0s, srclists, perm


def _prep_inputs(inputs):
    x = np.asarray(inputs["x"], np.float32)
    qhs, qls, idxs, idx0s, srclists, perm = _pack(
        inputs["edge_index"], inputs["edge_attr"])

    x_pad = np.zeros((NPAD, IN_F), np.float32)
    x_pad[perm] = x
    xt = np.ascontiguousarray(x_pad.T)                      # [32, NPAD]
    _, _, xh_u, xl_u = _bf16_split(xt)
    xs_full = np.concatenate([xh_u, xl_u], axis=0)          # [64, NPAD] u16
    xsos = [
        np.ascontiguousarray(xs_full[:, k * NC_COLS:(k + 1) * NC_COLS])
        for k in range(NCORES)
    ]
    xs0s = []
    for k in range(NCORES):
        xs0 = np.zeros((64, NSRC0), np.uint16)
        xs0[:, :srclists[k].size] = xs_full[:, srclists[k]]
        xs0s.append(xs0)

    def stack2(a):                                          # [m,n] -> [2m,n]
        return np.concatenate([a, a], axis=0)

    lin0_w = np.asarray(inputs["lin0_w"], np.float32)       # [32, 64]
    _, _, w0h, w0l = _bf16_split(lin0_w)
    l0 = np.zeros((64, 128), np.uint16)
    l0[:, 0:64] = stack2(w0h)
    l0[:, 64:128] = stack2(w0l)

    nw = np.asarray(inputs["nn_w"], np.float32)
    ws = np.zeros((128, NCH * 128), np.uint16)
    for c in range(NCH):
        wc = (nw[c].reshape(H, H) if c < 4
              else np.asarray(inputs["nn_b"], np.float32).reshape(H, H))
        _, _, wch, wcl = _bf16_split(wc)
        ws[:, c * 128:c * 128 + 64] = stack2(wch)
        ws[:, c * 128 + 64:c * 128 + 128] = stack2(wcl)

    root_w = np.asarray(inputs["root_w"], np.float32)
    _, _, rh, rl = _bf16_split(root_w)
    root = np.concatenate([rh, rl], axis=0)                 # [128, 64]

    wih_t = np.ascontiguousarray(np.asarray(inputs["gru_w_ih"], np.float32).T)
    whh_t = np.ascontiguousarray(np.asarray(inputs["gru_w_hh"], np.float32).T)
    _, _, wih_h, wih_l = _bf16_split(wih_t)                 # [64, 192]
    _, _, whh_h, whh_l = _bf16_split(whh_t)
    gru = np.zeros((128, 384), np.uint16)
    gru[0:64, 0:128] = wih_h[:, 0:128]                      # Vih_rz
    gru[64:128, 0:128] = wih_l[:, 0:128]
    gru[0:64, 128:256] = whh_h[:, 0:128]                    # Vhh_rz
    gru[64:128, 128:256] = whh_l[:, 0:128]
    gru[0:64, 256:320] = wih_h[:, 128:192]                  # Vih_n
    gru[64:128, 256:320] = wih_l[:, 128:192]
    gru[0:64, 320:384] = whh_h[:, 128:192]                  # Vhh_n
    gru[64:128, 320:384] = whh_l[:, 128:192]

    b_ih = np.asarray(inputs["gru_b_ih"], np.float32)
    b_hh = np.asarray(inputs["gru_b_hh"], np.float32)
    bias_pack = np.zeros((128, 8), np.float32)
    bias_pack[0:64, 0] = np.asarray(inputs["lin0_b"], np.float32)
    bias_pack[0:64, 1] = np.asarray(inputs["conv_b"], np.float32)
    bias_pack[0:64, 2] = (b_ih + b_hh)[0:64]                # b_r
    bias_pack[64:128, 2] = (b_ih + b_hh)[64:128]            # b_z
    bias_pack[0:64, 4] = b_ih[128:192]
    bias_pack[0:64, 5] = b_hh[128:192]
    identf = np.eye(64, dtype=np.float32)
    _, ident_u = _bf16_rne(np.eye(128, dtype=np.float32))

    in_maps = []
    for k in range(NCORES):
        in_maps.append(
            {
                "qh_in": qhs[k],
                "ql_in": qls[k],
                "idx_in": idxs[k],
                "idx0_in": idx0s[k],
                "xs0_in": xs0s[k],
                "xso_in": xsos[k],
                "l0_in": l0,
                "ws_in": ws,
                "root_in": root,
                "gru_in": gru,
                "bias_in": bias_pack,
                "ident_in": ident_u,
                "identf_in": identf,
            }
        )
    return in_maps, perm


def _assemble(results, perm):
    full = np.concatenate([results[k]["out_sl"] for k in range(NCORES)], axis=0)
    return np.ascontiguousarray(full[perm]).astype(np.float32)


def kernel(**inputs) -> np.ndarray:
    in_maps, perm = _prep_inputs(inputs)
    nc = _get_nc()
    if os.environ.get("BASS_KERNEL_SIM"):
        results = _run_sim(nc, in_maps)
    else:
        from concourse import bass_utils

        res = bass_utils.run_bass_kernel_spmd(
            nc, in_maps, core_ids=list(range(NCORES))
        )
        results = res.results
    return _assemble(results, perm)


def _run_sim(nc, in_maps):
    from concourse.bass_interp import MultiCoreSim

    sim = MultiCoreSim(nc, num_cores=NCORES, trace=False)
    for k, core in sim.cores.items():
        for name, arr in in_maps[k].items():
            core.tensor(name)[:] = arr
    sim.simulate(check_with_hw=False)
    out = []
    for k in range(NCORES):
        out.append({"out_sl": np.array(sim.cores[k].tensor("out_sl"))})
    return out


if __name__ == "__main__":
    rng = np.random.default_rng(0)
    demo = {
        "x": rng.standard_normal((N_NODES, IN_F), dtype=np.float32),
        "edge_index": rng.integers(0, N_NODES, (2, N_EDGES)).astype(np.int32),
        "edge_attr": rng.random((N_EDGES, 4), dtype=np.float32),
        "lin0_w": rng.standard_normal((IN_F, H), dtype=np.float32) * 0.1,
        "lin0_b": np.zeros(H, np.float32),
        "nn_w": rng.standard_normal((4, H * H), dtype=np.float32) * 0.05,
        "nn_b": np.zeros(H * H, np.float32),
        "root_w": rng.standard_normal((H, H), dtype=np.float32) * 0.1,
        "conv_b": np.zeros(H, np.float32),
        "gru_w_ih": rng.standard_normal((3 * H, H), dtype=np.float32) * 0.1,
        "gru_w_hh": rng.standard_normal((3 * H, H), dtype=np.float32) * 0.1,
        "gru_b_ih": np.zeros(3 * H, np.float32),
        "gru_b_hh": np.zeros(3 * H, np.float32),
    }
    out = kernel(**demo)
    print("kernel output", out.shape, out.dtype, float(np.abs(out).mean()))


# revision 29
# speedup vs baseline: 1.1587x; 1.0332x over previous
"""Bass/Trainium2 kernel for nn_Net_19602230739296 (NNConv + GRU message passing GNN).

Algorithm (mathematically equivalent to the reference):
  theta[e] = (edge_attr[e] @ nn_w + nn_b).reshape(H, H) is never materialized.
  msg[e]   = sum_c ea'[e,c] * (out[src_e] @ W_c)   with ea' = [edge_attr, 1],
             W_c = nn_w[c].reshape(H,H) for c<4, W_4 = nn_b.reshape(H,H).
  agg^T    = sum_c W_c^T @ (G^T @ Q_c)  per 128-edge tile, where G = out[src]
             (gathered rows) and Q_c[e, slot] = ea'[e,c] * [dst_e == slot-node]
             is a host-precomputed weighted one-hot "scatter" matrix.

Numerics: every fp32 value on the matmul paths is represented as a bf16
hi/lo pair (hi = bf16(x), lo = bf16(x - hi)).  bf16 matmuls run at 1 PE
cycle/row vs fp32's 4, and the PE multiplies bf16 exactly with fp32
accumulation, so a 3-term product (hi*hi + hi*lo + lo*hi) is accurate to
~2^-18 relative -- far inside the 2e-2 harness gate.  Node features live in
DRAM as [node, 128] rows = (hi 64 | lo 64) bf16, so one 256B-row gather
feeds the edge matmul with both terms and the per-tile matmul computes the
hi- and lo- partial products in one pass (128-partition PSUM output).

Sharding: edges are sorted by destination and packed into tiles of <=128
edges covering <=32 whole destination nodes.  Real tiles are dealt
round-robin across the 8 cores so each core gets an equal share of edges.
Nodes are renumbered to (tile*32 + slot).  A core's edges land only in its
own node range, so no cross-core reduction is needed.  The evolving node
features are replicated via AllGather each iteration (chunked, so the
collective overlaps the tail of the GRU); iteration 0's features are
computed for ALL nodes on every core (lin0 is tiny), which removes one
AllGather entirely.
"""
import os
import sys

import numpy as np


def _ensure_path():
    for p in ("/opt/trn_rl_repo", os.path.expanduser("~/.axon_site/_ro/trn_rl_repo")):
        if os.path.isdir(p) and p not in sys.path:
            sys.path.insert(0, p)
    try:
        import concourse  # noqa: F401
    except ImportError as e:  # pragma: no cover
        raise ImportError(f"concourse (bass) not importable: {e}")


_ensure_path()

N_NODES, N_EDGES, IN_F, H = 10000, 50000, 32, 64
NCORES = 8
SLOTS = 24            # destination-node slots per tile
EPT = 128             # edge slots per tile
NCH = 5               # edge_attr channels (4) + constant channel for nn_b
T = 64                # tiles per core (fixed so the compiled NEFF is shape-stable)
NTILES = NCORES * T   # 448
NC_COLS = T * SLOTS   # padded nodes per core (1792)
NPAD = NCORES * NC_COLS
CHUNK = 384
GATHER_CHUNKS = 16
N_SWDGE_QUEUES = 4
QW = NCH * SLOTS      # Q columns per tile
NSRC0 = 6144          # compact it-0 feature-table rows (unique sources, padded)
FB = 2 * H            # 128 bf16 feature bytes-row: hi|lo


def _chunks():
    out = []
    c0 = 0
    while c0 < NC_COLS:
        w = min(CHUNK, NC_COLS - c0)
        out.append((c0, w))
        c0 += w
    return out


# ----------------------------------------------------------------------------
# device program
# ----------------------------------------------------------------------------
_NC_CACHE = {}


def _get_nc():
    if "nc" in _NC_CACHE:
        return _NC_CACHE["nc"]
    import concourse.bacc as bacc
    import concourse.mybir as mybir
    import concourse.tile as tile

    dt = mybir.dt
    f32, i16, bf16 = dt.float32, dt.int16, dt.bfloat16
    AF = mybir.ActivationFunctionType
    ALU = mybir.AluOpType

    nc = bacc.Bacc(
        "TRN2",
        target_bir_lowering=False,
        debug=False,
        enable_asserts=False,
        num_devices=NCORES,
        num_swdge_queues=N_SWDGE_QUEUES,
    )

    qh_in = nc.dram_tensor("qh_in", [128, T * QW], bf16, kind="ExternalInput").ap()
    ql_in = nc.dram_tensor("ql_in", [128, T * QW], bf16, kind="ExternalInput").ap()
    idx_in = nc.dram_tensor("idx_in", [128, T * 8], i16, kind="ExternalInput").ap()
    xs0_in = nc.dram_tensor("xs0_in", [64, NSRC0], bf16, kind="ExternalInput").ap()
    idx0_in = nc.dram_tensor("idx0_in", [128, T * 8], i16, kind="ExternalInput").ap()
    xso_in = nc.dram_tensor("xso_in", [64, NC_COLS], bf16, kind="ExternalInput").ap()
    l0_in = nc.dram_tensor("l0_in", [64, 128], bf16, kind="ExternalInput").ap()
    ws_in = nc.dram_tensor("ws_in", [128, NCH * 128], bf16, kind="ExternalInput").ap()
    root_in = nc.dram_tensor("root_in", [128, 64], bf16, kind="ExternalInput").ap()
    gru_in = nc.dram_tensor("gru_in", [128, 384], bf16, kind="ExternalInput").ap()
    bias_in = nc.dram_tensor("bias_in", [128, 8], f32, kind="ExternalInput").ap()
    ident_in = nc.dram_tensor("ident_in", [128, 128], bf16, kind="ExternalInput").ap()
    identf_in = nc.dram_tensor("identf_in", [64, 64], f32, kind="ExternalInput").ap()
    out_ext = nc.dram_tensor("out_sl", [NC_COLS, H], f32, kind="ExternalOutput").ap()

    chunks = _chunks()

    with tile.TileContext(nc) as tc:
        with tc.tile_pool(name="const", bufs=1) as const, \
             tc.tile_pool(name="work", bufs=1) as work, \
             tc.tile_pool(name="small", bufs=2) as small, \
             tc.tile_pool(name="ps", bufs=2, space="PSUM") as ps, \
             tc.tile_pool(name="aggp", bufs=2, space="PSUM") as aggp, \
             tc.tile_pool(name="grz", bufs=1, space="PSUM") as grz, \
             tc.tile_pool(name="gn", bufs=2, space="PSUM") as gn, \
             tc.tile_pool(name="tpp", bufs=1, space="PSUM") as tpp, \
             tc.tile_pool(name="dram", bufs=1, space="DRAM") as dram:

            qh_sb = const.tile([128, T * QW], bf16, name="qh_sb")
            ql_sb = const.tile([128, T * QW], bf16, name="ql_sb")
            idx_sb = const.tile([128, T * 8], i16, name="idx_sb")
            xs0_sb = const.tile([64, NSRC0], bf16, name="xs0_sb")
            idx0_sb = const.tile([128, T * 8], i16, name="idx0_sb")
            xso_sb = const.tile([64, NC_COLS], bf16, name="xso_sb")
            l0_sb = const.tile([64, 128], bf16, name="l0_sb")
            ws_sb = const.tile([128, NCH * 128], bf16, name="ws_sb")
            root_sb = const.tile([128, 64], bf16, name="root_sb")
            gru_sb = const.tile([128, 384], bf16, name="gru_sb")
            bias_sb = const.tile([128, 8], f32, name="bias_sb")
            ident_sb = const.tile([128, 128], bf16, name="ident_sb")
            identf_sb = const.tile([64, 64], f32, name="identf_sb")

            for sb_t, in_t in (
                (xso_sb, xso_in), (l0_sb, l0_in), (bias_sb, bias_in),
                (ident_sb, ident_in), (identf_sb, identf_in),
                (xs0_sb, xs0_in), (idx0_sb, idx0_in), (idx_sb, idx_in),
                (gru_sb, gru_in), (root_sb, root_in), (ws_sb, ws_in),
                (qh_sb, qh_in), (ql_sb, ql_in),
            ):
                nc.sync.dma_start(sb_t[:], in_t[:])

            l0b = bias_sb[0:64, 0:1]
            convb = bias_sb[0:64, 1:2]
            brz = bias_sb[:, 2:3]          # [br ; bz] stacked on 128 partitions
            bnih = bias_sb[0:64, 4:5]
            bnhh = bias_sb[0:64, 5:6]

            h32a = work.tile([128, NC_COLS], f32, name="h32a")
            h32b = work.tile([128, NC_COLS], f32, name="h32b")
            # h bf16 split, stacked [hi;lo] and swapped [lo;hi] on 128 parts
            hs1a = work.tile([128, NC_COLS], bf16, name="hs1a")
            hs2a = work.tile([128, NC_COLS], bf16, name="hs2a")
            hs1b = work.tile([128, NC_COLS], bf16, name="hs1b")
            hs2b = work.tile([128, NC_COLS], bf16, name="hs2b")
            g_sb = work.tile([128, T * FB], bf16, name="g_sb")
            sh_sb = work.tile([128, T * QW], bf16, name="sh_sb")
            sl_sb = work.tile([128, T * QW], bf16, name="sl_sb")
            rowb = work.tile([128, NC_COLS], bf16, name="rowb")
            rowf = work.tile([128, (NC_COLS // 128) * H], f32, name="rowf")
            warm_sb = work.tile([128, 64], bf16, name="warm_sb")

            feat0 = dram.tile([NSRC0, FB], bf16, name="feat0")
            agins = [dram.tile([NC_COLS, FB], bf16, name=f"agin{i}") for i in (1, 2)]
            agouts = [
                dram.tile([NPAD, FB], bf16, addr_space="Shared", name=f"agout{i}")
                for i in (1, 2)
            ]

            # ---- iteration 0: out0 = relu(x @ lin0_w + lin0_b) --------------
            # Full-table pass: every core computes out0 for ALL nodes and
            # writes the bf16 hi|lo feature table locally -- no collective.
            for c in range(NSRC0 // CHUNK):
                sl = slice(c * CHUNK, (c + 1) * CHUNK)
                p0 = gn.tile([64, CHUNK], f32, tag="gn", name=f"l0f{c}")
                nc.tensor.matmul(p0[:], l0_sb[:, 0:64], xs0_sb[:, sl],
                                 start=True, stop=False)
                nc.tensor.matmul(p0[:], l0_sb[:, 64:128], xs0_sb[:, sl],
                                 start=False, stop=True)
                o32 = small.tile([128, CHUNK], f32, tag="o32", name=f"o32_{c}")
                nc.scalar.activation(o32[0:64, :], p0[:], AF.Relu, bias=l0b)
                osk = small.tile([128, CHUNK], bf16, tag="osk", name=f"osk_{c}")
                nc.vector.tensor_copy(osk[0:64, :], o32[0:64, :])
                nc.vector.tensor_sub(osk[64:128, :], o32[0:64, :], osk[0:64, :])
                tp = tpp.tile([128, 512], bf16, tag="tp", name=f"t0_{c}")
                for b in range(CHUNK // 128):
                    bs = slice(b * 128, (b + 1) * 128)
                    nc.tensor.transpose(tp[:, b * 128:(b + 1) * 128],
                                        osk[:, bs], ident_sb[:])
                nbc = CHUNK // 128
                rf = small.tile([128, CHUNK], bf16, tag="rf", name=f"rf_{c}")
                if c % 2 == 0:
                    nc.scalar.copy(rf[:], tp[:, :CHUNK])
                else:
                    nc.vector.tensor_copy(rf[:], tp[:, :CHUNK])
                nc.sync.dma_start(
                    feat0[:].rearrange("(t p) f -> p t f", p=128)[
                        :, nbc * c:nbc * c + nbc, :],
                    rf[:].rearrange("p (t f) -> p t f", f=FB),
                )
            # Own-slice pass (fp32 h and its bf16 split for root/GRU inputs).
            for k, (c0, w) in enumerate(chunks):
                sl = slice(c0, c0 + w)
                p1 = gn.tile([64, CHUNK], f32, tag="gn", name=f"l0o{k}")
                nc.tensor.matmul(p1[:, :w], l0_sb[:, 0:64], xso_sb[:, sl],
                                 start=True, stop=False)
                nc.tensor.matmul(p1[:, :w], l0_sb[:, 64:128], xso_sb[:, sl],
                                 start=False, stop=True)
                nc.scalar.activation(h32a[0:64, sl], p1[:, :w], AF.Relu, bias=l0b)
                nc.vector.tensor_copy(hs1a[0:64, sl], h32a[0:64, sl])
                nc.vector.tensor_sub(hs1a[64:128, sl], h32a[0:64, sl],
                                     hs1a[0:64, sl])
                nc.scalar.copy(hs2a[0:64, sl], hs1a[64:128, sl])
                nc.scalar.copy(hs2a[64:128, sl], hs1a[0:64, sl])

            def warm_chain(links, it):
                # Keep the PE's HAM activity monitor from dropping to the
                # cold 1.2GHz clock while the engine waits on the collective:
                # a dependency chain of tiny matmuls paces ~1 PE op per µs.
                for i in range(links):
                    wp = gn.tile([64, CHUNK], f32, tag="gn", name=f"wm{it}_{i}")
                    nc.tensor.matmul(wp[:, 0:64], ident_sb[:, 0:64],
                                     warm_sb[:], start=True, stop=True)
                    nc.vector.tensor_copy(warm_sb[0:64, :], wp[:, 0:64])
                    nc.vector.tensor_copy(warm_sb[64:128, :], wp[:, 0:64])

            def edge_phase(it):
                src_dram = feat0 if it == 1 else agouts[it - 2]
                idxt = idx0_sb if it == 1 else idx_sb
                gsz = T // GATHER_CHUNKS
                for gc in range(GATHER_CHUNKS):
                    nc.gpsimd.dma_gather(
                        g_sb[:, gc * gsz * FB:(gc + 1) * gsz * FB].rearrange(
                            "p (t o) -> p t o", o=FB
                        ),
                        src_dram[:],
                        idxt[:, gc * gsz * 8:(gc + 1) * gsz * 8],
                        gsz * EPT,
                        gsz * EPT,
                        FB,
                        queue_num=gc % N_SWDGE_QUEUES,
                    )
                # mm1: S = G^T @ (Qh + Ql) per tile; PSUM partitions 0:64 get
                # the hi-feature products, 64:128 the lo-feature products.
                t = 0
                while t < T:
                    ntl = min(3, T - t)
                    s_ps = ps.tile([128, 3 * QW], f32, tag="s", name=f"sps{it}_{t}")
                    for j in range(ntl):
                        nc.tensor.matmul(
                            s_ps[:, j * QW:(j + 1) * QW],
                            g_sb[:, (t + j) * FB:(t + j + 1) * FB],
                            qh_sb[:, (t + j) * QW:(t + j + 1) * QW],
                            start=True, stop=False,
                        )
                        nc.tensor.matmul(
                            s_ps[:, j * QW:(j + 1) * QW],
                            g_sb[:, (t + j) * FB:(t + j + 1) * FB],
                            ql_sb[:, (t + j) * QW:(t + j + 1) * QW],
                            start=False, stop=True,
                        )
                    if (t // 3) % 2 == 0:
                        nc.scalar.copy(sh_sb[:, t * QW:(t + ntl) * QW],
                                       s_ps[:, :ntl * QW])
                    else:
                        nc.vector.tensor_copy(sh_sb[:, t * QW:(t + ntl) * QW],
                                              s_ps[:, :ntl * QW])
                    nc.vector.tensor_sub(sl_sb[:, t * QW:(t + ntl) * QW],
                                         s_ps[:, :ntl * QW],
                                         sh_sb[:, t * QW:(t + ntl) * QW])
                    t += ntl

            sh_re = sh_sb[:].rearrange("p (t c s) -> p t c s", c=NCH, s=SLOTS)
            sl_re = sl_sb[:].rearrange("p (t c s) -> p t c s", c=NCH, s=SLOTS)

            def dense_chunk(it, k, c0, w, h32, hs1, hs2, hn32, hs1n, hs2n):
                sl = slice(c0, c0 + w)
                t0, tn = c0 // SLOTS, w // SLOTS
                # mm2 (+root) for this chunk's 16 tiles
                agg = aggp.tile([64, CHUNK], f32, tag="agg", name=f"agg{it}_{k}")
                for c in range(NCH):
                    nc.tensor.matmul(agg[:, :w], ws_sb[:, c * 128:c * 128 + 64],
                                     sh_re[:, t0:t0 + tn, c, :],
                                     start=(c == 0), stop=False)
                    nc.tensor.matmul(agg[:, :w], ws_sb[:, c * 128:c * 128 + 64],
                                     sl_re[:, t0:t0 + tn, c, :],
                                     start=False, stop=False)
                    nc.tensor.matmul(agg[:, :w], ws_sb[:, c * 128 + 64:c * 128 + 128],
                                     sh_re[:, t0:t0 + tn, c, :],
                                     start=False, stop=False)
                nc.tensor.matmul(agg[:, :w], root_sb[:], hs1[:, sl],
                                 start=False, stop=False)
                nc.tensor.matmul(agg[:, :w], root_sb[:], hs2[:, sl],
                                 start=False, stop=True)
                # m = relu(agg + conv_b); bf16 stacks [mh;ml] and [ml;mh]
                m32 = small.tile([128, CHUNK], f32, tag="m32", name=f"m32_{it}{k}")
                nc.scalar.activation(m32[0:64, :w], agg[:, :w], AF.Relu, bias=convb)
                ms1 = small.tile([128, CHUNK], bf16, tag="ms1", name=f"ms1_{it}{k}")
                ms2 = small.tile([128, CHUNK], bf16, tag="ms2", name=f"ms2_{it}{k}")
                nc.vector.tensor_copy(ms1[0:64, :w], m32[0:64, :w])
                nc.vector.tensor_sub(ms1[64:128, :w], m32[0:64, :w], ms1[0:64, :w])
                nc.scalar.copy(ms2[0:64, :w], ms1[64:128, :w])
                nc.scalar.copy(ms2[64:128, :w], ms1[0:64, :w])

                # r|z stacked on 128 partitions: 4 FWL matmuls
                rz = grz.tile([128, CHUNK], f32, tag="rz", name=f"rz{it}_{k}")
                nc.tensor.matmul(rz[:, :w], gru_sb[:, 0:128], ms1[:, :w],
                                 start=True, stop=False)
                nc.tensor.matmul(rz[:, :w], gru_sb[:, 0:128], ms2[:, :w],
                                 start=False, stop=False)
                nc.tensor.matmul(rz[:, :w], gru_sb[:, 128:256], hs1[:, sl],
                                 start=False, stop=False)
                nc.tensor.matmul(rz[:, :w], gru_sb[:, 128:256], hs2[:, sl],
                                 start=False, stop=True)
                rz_sb = small.tile([128, CHUNK], f32, tag="rzsb", name=f"rzs{it}{k}")
                nc.scalar.activation(rz_sb[:, :w], rz[:, :w], AF.Sigmoid, bias=brz)
                n1 = gn.tile([64, CHUNK], f32, tag="gn", name=f"n1{it}_{k}")
                nc.tensor.matmul(n1[:, :w], gru_sb[:, 256:320], ms1[:, :w],
                                 start=True, stop=False)
                nc.tensor.matmul(n1[:, :w], gru_sb[:, 256:320], ms2[:, :w],
                                 start=False, stop=True)
                n2 = gn.tile([64, CHUNK], f32, tag="gn", name=f"n2{it}_{k}")
                nc.tensor.matmul(n2[:, :w], gru_sb[:, 320:384], hs1[:, sl],
                                 start=True, stop=False)
                nc.tensor.matmul(n2[:, :w], gru_sb[:, 320:384], hs2[:, sl],
                                 start=False, stop=True)
                # tmp = (n2 + b_hh_n) * r
                tmp = small.tile([64, CHUNK], f32, tag="tmp", name=f"tmp{it}{k}")
                nc.vector.scalar_tensor_tensor(
                    tmp[:, :w], n2[:, :w], bnhh, rz_sb[0:64, :w], ALU.add, ALU.mult
                )
                pre = small.tile([64, CHUNK], f32, tag="pre", name=f"pre{it}{k}")
                nc.vector.tensor_add(pre[:, :w], n1[:, :w], tmp[:, :w])
                nsb = small.tile([128, CHUNK], f32, tag="nsb", name=f"nsb{it}{k}")
                nc.scalar.activation(nsb[0:64, :w], pre[:, :w], AF.Tanh, bias=bnih)
                # h' = n + z * (h - n)
                dd = small.tile([128, CHUNK], f32, tag="dd", name=f"dd{it}{k}")
                nc.vector.tensor_sub(dd[0:64, :w], h32[0:64, sl], nsb[0:64, :w])
                t4 = small.tile([128, CHUNK], f32, tag="t4", name=f"t4{it}{k}")
                z_sb = small.tile([128, CHUNK], f32, tag="zsb", name=f"zsb{it}{k}")
                nc.scalar.copy(z_sb[0:64, :w], rz_sb[64:128, :w])
                nc.vector.tensor_mul(t4[0:64, :w], z_sb[0:64, :w], dd[0:64, :w])
                nc.vector.tensor_add(hn32[0:64, sl], nsb[0:64, :w], t4[0:64, :w])
                if it < 3:
                    # bf16 split (and swap) of the new h for the next iteration
                    nc.scalar.copy(hs1n[0:64, sl], hn32[0:64, sl])
                    nc.vector.tensor_sub(hs1n[64:128, sl], hn32[0:64, sl],
                                         hs1n[0:64, sl])
                    nc.sync.dma_start(hs2n[0:64, sl], hs1n[64:128, sl])
                    nc.sync.dma_start(hs2n[64:128, sl], hs1n[0:64, sl])

            def writeback_chunk(it, k, c0, w, hs1n, hn32):
                nb = w // 128
                if it < 3:
                    tp = tpp.tile([128, 512], bf16, tag="tp", name=f"w{it}_{k}")
                    for b in range(nb):
                        bs = slice(c0 + b * 128, c0 + (b + 1) * 128)
                        nc.tensor.transpose(tp[:, b * 128:(b + 1) * 128],
                                            hs1n[:, bs], ident_sb[:])
                    if k % 2 == 0:
                        nc.scalar.copy(rowb[:, c0:c0 + w], tp[:, :nb * 128])
                    else:
                        nc.vector.tensor_copy(rowb[:, c0:c0 + w], tp[:, :nb * 128])
                    agin, agout = agins[it - 1], agouts[it - 1]
                    nc.sync.dma_start(
                        agin[:].rearrange("(t p) f -> p t f", p=128)[
                            :, c0 // 128:c0 // 128 + nb, :],
                        rowb[:, c0:c0 + w].rearrange("p (t f) -> p t f", f=FB),
                    )
                    if c0 + w == NC_COLS:
                        # Shared DRAM allows a single writer instruction, so
                        # the AllGather fires once, after the last chunk DMA.
                        nc.gpsimd.collective_compute(
                            "AllGather",
                            mybir.AluOpType.bypass,
                            replica_groups=[list(range(NCORES))],
                            ins=[agin[:]],
                            outs=[agout[:]],
                        )
                else:
                    # final iteration: write fp32 h to the external output
                    tpf = tpp.tile([128, 256], f32, tag="tp", name=f"wf_{k}")
                    for b in range(nb):
                        bs = slice(c0 + b * 128, c0 + (b + 1) * 128)
                        nc.tensor.transpose(
                            tpf[:, b * 64:(b + 1) * 64], hn32[0:64, bs],
                            identf_sb[:])
                    if k % 2 == 0:
                        nc.scalar.copy(rowf[:, (c0 // 128) * H:(c0 // 128 + nb) * H],
                                       tpf[:, :nb * 64])
                    else:
                        nc.vector.tensor_copy(
                            rowf[:, (c0 // 128) * H:(c0 // 128 + nb) * H],
                            tpf[:, :nb * 64])

            nc.vector.tensor_copy(warm_sb[:], ident_sb[:, 0:64])
            h32, hn32 = h32a, h32b
            cur = (hs1a, hs2a, hs1b, hs2b)
            for it in (1, 2, 3):
                hs1, hs2, hs1n, hs2n = cur
                warm_chain(8 if it == 1 else 80, it)
                edge_phase(it)
                for k, (c0, w) in enumerate(chunks):
                    dense_chunk(it, k, c0, w, h32, hs1, hs2, hn32, hs1n, hs2n)
                    writeback_chunk(it, k, c0, w, hs1n, hn32)
                h32, hn32 = hn32, h32
                cur = (hs1n, hs2n, hs1, hs2)

            nc.sync.dma_start(
                out_ext[:].rearrange("(t p) o -> p t o", p=128),
                rowf[:].rearrange("p (t o) -> p t o", o=H),
            )

    nc.compile()
    _NC_CACHE["nc"] = nc
    return nc


# ----------------------------------------------------------------------------
# host-side graph preprocessing (pure data layout, no model FLOPs)
# ----------------------------------------------------------------------------
def _bf16_rne(x):
    """Round fp32 -> bf16 (round-to-nearest-even). Returns (f32val, uint16bits)."""
    u = np.asarray(x, np.float32).view(np.uint32)
    r = (u + 0x7FFF + ((u >> 16) & 1)) & 0xFFFF0000
    return r.view(np.float32), (r >> 16).astype(np.uint16)


def _bf16_split(x):
    """x ~= hi + lo with both bf16. Returns (hi_f32, lo_f32, hi_u16, lo_u16)."""
    x = np.ascontiguousarray(np.asarray(x, np.float32))
    hi_f, hi_u = _bf16_rne(x)
    lo_f, lo_u = _bf16_rne(x - hi_f)
    return hi_f, lo_f, hi_u, lo_u


def _pack(edge_index, edge_attr):
    src = np.asarray(edge_index[0]).astype(np.int64)
    dst = np.asarray(edge_index[1]).astype(np.int64)
    ea = np.asarray(edge_attr, np.float32)
    order = np.argsort(dst, kind="stable")
    ssrc, sea = src[order], ea[order]
    deg = np.bincount(dst, minlength=N_NODES)
    starts = np.zeros(N_NODES + 1, np.int64)
    starts[1:] = np.cumsum(deg)
    uniq = np.flatnonzero(deg)
    zs = np.flatnonzero(deg == 0)
    node_seq = np.concatenate([uniq, zs])

    raw_tiles = [[]]
    ce = 0
    for nd in node_seq:
        d = int(deg[nd])
        assert d <= EPT, f"node degree {d} exceeds edge tile capacity"
        if len(raw_tiles[-1]) >= SLOTS or ce + d > EPT:
            raw_tiles.append([])
            ce = 0
        raw_tiles[-1].append(int(nd))
        ce += d
    assert len(raw_tiles) <= NTILES, f"need {len(raw_tiles)} tiles > {NTILES}"
    # Distribute real tiles round-robin across the 8 cores so every core gets
    # an equal share of real edges (a contiguous split leaves the last core
    # nearly all padding, which skews its runtime and stalls the collectives).
    tiles_nodes = [[] for _ in range(NTILES)]
    for i, nodes in enumerate(raw_tiles):
        core, j = i % NCORES, i // NCORES
        tiles_nodes[core * T + j] = nodes

    perm = np.empty(N_NODES, np.int64)
    for t, nodes in enumerate(tiles_nodes):
        for j, nd in enumerate(nodes):
            perm[nd] = t * SLOTS + j

    q = np.zeros((NTILES, EPT, NCH, SLOTS), np.float32)
    # Padding gather slots must NOT all point at row 0: thousands of reads of
    # one 256B row serialize on a single HBM bank.  Padding gets spread
    # distinct rows in the core's own slice.
    srcslot = np.full((NTILES, EPT), -1, np.int16)
    for t, nodes in enumerate(tiles_nodes):
        e = 0
        for j, nd in enumerate(nodes):
            s0, s1 = int(starts[nd]), int(starts[nd + 1])
            ne = s1 - s0
            if ne:
                q[t, e:e + ne, 0:4, j] = sea[s0:s1]
                q[t, e:e + ne, 4, j] = 1.0
                srcslot[t, e:e + ne] = perm[ssrc[s0:s1]].astype(np.int16)
                e += ne
    for k in range(NCORES):
        base = k * NC_COLS
        block = srcslot[k * T:(k + 1) * T].reshape(-1)
        holes = np.flatnonzero(block < 0)
        block[holes] = base + np.arange(holes.size) % NC_COLS
        srcslot[k * T:(k + 1) * T] = block.reshape(T, EPT)

    _, _, qh_u, ql_u = _bf16_split(q)
    qhs, qls, idxs, idx0s, srclists = [], [], [], [], []
    i_arange = np.arange(T * EPT)

    def wrap16(flat):
        # the index list is read per 16-partition group by each of the 8
        # GPSIMD cores on HW -> replicate it into every group
        ia = np.zeros((128, T * 8), np.int16)
        for g in range(8):
            ia[g * 16 + i_arange % 16, i_arange // 16] = flat
        return ia

    for k in range(NCORES):
        def qlay(qu):
            qt = qu[k * T:(k + 1) * T]
            return np.ascontiguousarray(qt.transpose(1, 0, 2, 3)).reshape(
                128, T * QW)
        qhs.append(qlay(qh_u.reshape(NTILES, EPT, NCH, SLOTS)))
        qls.append(qlay(ql_u.reshape(NTILES, EPT, NCH, SLOTS)))
        flat = srcslot[k * T:(k + 1) * T].reshape(-1).astype(np.int64)
        idxs.append(wrap16(flat.astype(np.int16)))
        # compact source space for iteration 1: only the rows this core
        # actually gathers exist in its local feat0 table
        srcs, inv = np.unique(flat, return_inverse=True)
        assert srcs.size <= NSRC0, f"core {k}: {srcs.size} > {NSRC0}"
        srclists.append(srcs)
        idx0s.append(wrap16(inv.astype(np.int16)))
    return qhs, qls, idxs, idx0s, srclists, perm


def _prep_inputs(inputs):
    x = np.asarray(inputs["x"], np.float32)
    qhs, qls, idxs, idx0s, srclists, perm = _pack(
        inputs["edge_index"], inputs["edge_attr"])

    x_pad = np.zeros((NPAD, IN_F), np.float32)
    x_pad[perm] = x
    xt = np.ascontiguousarray(x_pad.T)                      # [32, NPAD]
    _, _, xh_u, xl_u = _bf16_split(xt)
    xs_full = np.concatenate([xh_u, xl_u], axis=0)          # [64, NPAD] u16
    xsos = [
        np.ascontiguousarray(xs_full[:, k * NC_COLS:(k + 1) * NC_COLS])
        for k in range(NCORES)
    ]
    xs0s = []
    for k in range(NCORES):
        xs0 = np.zeros((64, NSRC0), np.uint16)
        xs0[:, :srclists[k].size] = xs_full[:, srclists[k]]
        xs0s.append(xs0)

    def stack2(a):                                          # [m,n] -> [2m,n]
        return np.concatenate([a, a], axis=0)

    lin0_w = np.asarray(inputs["lin0_w"], np.float32)       # [32, 64]
    _, _, w0h, w0l = _bf16_split(lin0_w)
    l0 = np.zeros((64, 128), np.uint16)
    l0[:, 0:64] = stack2(w0h)
    l0[:, 64:128] = stack2(w0l)

    nw = np.asarray(inputs["nn_w"], np.float32)
    ws = np.zeros((128, NCH * 128), np.uint16)
    for c in range(NCH):
        wc = (nw[c].reshape(H, H) if c < 4
              else np.asarray(inputs["nn_b"], np.float32).reshape(H, H))
        _, _, wch, wcl = _bf16_split(wc)
        ws[:, c * 128:c * 128 + 64] = stack2(wch)
        ws[:, c * 128 + 64:c * 128 + 128] = stack2(wcl)

    root_w = np.asarray(inputs["root_w"], np.float32)
    _, _, rh, rl = _bf16_split(root_w)
    root = np.concatenate([rh, rl], axis=0)                 # [128, 64]

    wih_t = np.ascontiguousarray(np.asarray(inputs["gru_w_ih"], np.float32).T)
    whh_t = np.ascontiguousarray(np.asarray(inputs["gru_w_hh"], np.float32).T)
    _, _, wih_h, wih_l = _bf16_split(wih_t)                 # [64, 192]
    _, _, whh_h, whh_l = _bf16_split(whh_t)
    gru = np.zeros((128, 384), np.uint16)
    gru[0:64, 0:128] = wih_h[:, 0:128]                      # Vih_rz
    gru[64:128, 0:128] = wih_l[:, 0:128]
    gru[0:64, 128:256] = whh_h[:, 0:128]                    # Vhh_rz
    gru[64:128, 128:256] = whh_l[:, 0:128]
    gru[0:64, 256:320] = wih_h[:, 128:192]                  # Vih_n
    gru[64:128, 256:320] = wih_l[:, 128:192]
    gru[0:64, 320:384] = whh_h[:, 128:192]                  # Vhh_n
    gru[64:128, 320:384] = whh_l[:, 128:192]

    b_ih = np.asarray(inputs["gru_b_ih"], np.float32)
    b_hh = np.asarray(inputs["gru_b_hh"], np.float32)
    bias_pack = np.zeros((128, 8), np.float32)
    bias_pack[0:64, 0] = np.asarray(inputs["lin0_b"], np.float32)
    bias_pack[0:64, 1] = np.asarray(inputs["conv_b"], np.float32)
    bias_pack[0:64, 2] = (b_ih + b_hh)[0:64]                # b_r
    bias_pack[64:128, 2] = (b_ih + b_hh)[64:128]            # b_z
    bias_pack[0:64, 4] = b_ih[128:192]
    bias_pack[0:64, 5] = b_hh[128:192]
    identf = np.eye(64, dtype=np.float32)
    _, ident_u = _bf16_rne(np.eye(128, dtype=np.float32))

    in_maps = []
    for k in range(NCORES):
        in_maps.append(
            {
                "qh_in": qhs[k],
                "ql_in": qls[k],
                "idx_in": idxs[k],
                "idx0_in": idx0s[k],
                "xs0_in": xs0s[k],
                "xso_in": xsos[k],
                "l0_in": l0,
                "ws_in": ws,
                "root_in": root,
                "gru_in": gru,
                "bias_in": bias_pack,
                "ident_in": ident_u,
                "identf_in": identf,
            }
        )
    return in_maps, perm


def _assemble(results, perm):
    full = np.concatenate([results[k]["out_sl"] for k in range(NCORES)], axis=0)
    return np.ascontiguousarray(full[perm]).astype(np.float32)


def kernel(**inputs) -> np.ndarray:
    in_maps, perm = _prep_inputs(inputs)
    nc = _get_nc()
    if os.environ.get("BASS_KERNEL_SIM"):
        results = _run_sim(nc, in_maps)
    else:
        from concourse import bass_utils

        res = bass_utils.run_bass_kernel_spmd(
            nc, in_maps, core_ids=list(range(NCORES))
        )
        results = res.results
    return _assemble(results, perm)


def _run_sim(nc, in_maps):
    from concourse.bass_interp import MultiCoreSim

    sim = MultiCoreSim(nc, num_cores=NCORES, trace=False)
    for k, core in sim.cores.items():
        for name, arr in in_maps[k].items():
            core.tensor(name)[:] = arr
    sim.simulate(check_with_hw=False)
    out = []
    for k in range(NCORES):
        out.append({"out_sl": np.array(sim.cores[k].tensor("out_sl"))})
    return out


if __name__ == "__main__":
    rng = np.random.default_rng(0)
    demo = {
        "x": rng.standard_normal((N_NODES, IN_F), dtype=np.float32),
        "edge_index": rng.integers(0, N_NODES, (2, N_EDGES)).astype(np.int32),
        "edge_attr": rng.random((N_EDGES, 4), dtype=np.float32),
        "lin0_w": rng.standard_normal((IN_F, H), dtype=np.float32) * 0.1,
        "lin0_b": np.zeros(H, np.float32),
        "nn_w": rng.standard_normal((4, H * H), dtype=np.float32) * 0.05,
        "nn_b": np.zeros(H * H, np.float32),
        "root_w": rng.standard_normal((H, H), dtype=np.float32) * 0.1,
        "conv_b": np.zeros(H, np.float32),
        "gru_w_ih": rng.standard_normal((3 * H, H), dtype=np.float32) * 0.1,
        "gru_w_hh": rng.standard_normal((3 * H, H), dtype=np.float32) * 0.1,
        "gru_b_ih": np.zeros(3 * H, np.float32),
        "gru_b_hh": np.zeros(3 * H, np.float32),
    }
    out = kernel(**demo)
    print("kernel output", out.shape, out.dtype, float(np.abs(out).mean()))
